# revision 4
# baseline (speedup 1.0000x reference)
"""MiniBatchDiscrimination Trainium2 kernel — DVE-fused-norm version (v3).

reference:
    M = einsum('nhwf,fbc->nhwbc', x, T)          # [N,H,W,B,C]
    norm = sum_c |M[i] - M[j]|                   # [N,N,H,W,B]
    o_b  = sum_j exp(-norm)                      # [N,H,W,B]
    out  = concat([x, o_b], axis=3)              # [N,H,W,F+B]

Sharding: embarrassingly parallel over HW=256 spatial positions; each core
takes a 32-position hw slice and computes all pairs for it.

Layout (per core): the c-axis lives INNERMOST IN THE FREE DIM so a custom
8-state DVE micro-op ("PAIRSUM8", 2X_1PORT) computes, per (i,hwl,b) position,
the full L1 norm sum_c |M2[i+d] - M2[i]| in a single streaming pass —
consuming 2 f16/lane/cycle and emitting one compacted f16 norm per 8 inputs.
This removes the PE c-reduce matmuls entirely.

  M2   [128 part=(hh2,b64), 6144 free=(i48,hl16,c8)] f16; i in [32,48) is a
       circular pad (copy of i in [0,16)) so every d-stream is contiguous.
  nrm  [128, 7936 free=(d15,i32,hl16 | d16:i16,hl16)] f16 — ONE giant DVE
       instruction covers d=1..15 via a 3D AP (outer dim d, in1 broadcast).
  E    exp(-nrm) f16 (ACT)
  o_ps [128 part=(hh,b), 512=(i,hl)] f32 PSUM — j-sum via identity-weight
       matmuls: direct (col) + shifted (col+16d mod 512) accumulation.
  Pair symmetry: stream d covers pairs (i, (i+d)%32); exp(-norm) folds into
  row i (direct) and row i+d (shifted); diagonal +1 fused into the drain.
"""

import os
import sys

for _p in ("/opt/trn_rl_repo", "/opt/pypackages"):
    if _p not in sys.path and os.path.isdir(_p):
        sys.path.append(_p)

import numpy as np

N, HWL, F, B, C = 32, 32, 256, 64, 8
HW = 256
CORES = 8
FH = 2           # f in two partition halves of 128
HH = 2           # hwl_hi: hw position high bit (partition dim)
HL = 16          # hwl_lo: 16 positions (free dim)
IPAD = 48        # i padded 32 -> 48 for circular d-shift reads

F16 = "float16"

_CACHED = {}


# --------------------------------------------------------------------------
# PAIRSUM8 custom DVE op: out[g] = sum_{k<8} |in0[8g+k] - in1[8g+k]|
# 2X_1PORT 8-state machine; emits two group sums (WR0_LO/WR0_HI) every
# 4th pair-cycle; out FD = in FD / 8, contiguous.
# --------------------------------------------------------------------------

def _mk_ps8_2x():
    from concourse.dve_uop import (
        ENABLE, AluInp, AluOp, DelayInp, InpSel, OutPath, OutSel, Trigger,
        UopConfig, UopDpConfig,
    )

    def base(next_idx):
        u = UopConfig()
        u.enable_input(InpSel.SRC_0, 0).enable_input(InpSel.SRC_1, 1)
        u.enable_input(InpSel.SRC_0_HI, 2).enable_input(InpSel.SRC_1_HI, 3)
        u.enable_input(InpSel.ZERO, 4)
        u.require_inp0 = ENABLE
        u.require_inp1 = ENABLE
        u.trigger = (Trigger.SRC_TENSOR_DONE, Trigger.COUNT, Trigger.NONE)
        u.next_uop = (0, next_idx, 0)
        u.repeat_count = 1
        dp = u.datapath_config
        # s0: |a_lo-b_lo|; carry a_hi(d1), b_hi(d2), zero(d3)
        dp[0] = (UopDpConfig()
                 .enable_alu(AluOp.ABSOLUTE_DIFF, AluInp.PREV_ALU_OUT,
                             AluInp.PREV_DELAY_0)
                 .pass_through_delay(1, 2, 3))
        # s1: |a_hi-b_hi|; d0 <- lo result
        dp[1] = (UopDpConfig()
                 .enable_alu(AluOp.ABSOLUTE_DIFF, AluInp.PREV_DELAY_1,
                             AluInp.PREV_DELAY_2)
                 .enable_delay_from_src(DelayInp.PREV_ALU_OUT, 0)
                 .pass_through_delay(3))
        # s2: pair sum s = lo + hi
        dp[2] = (UopDpConfig()
                 .enable_alu(AluOp.ADD, AluInp.PREV_ALU_OUT,
                             AluInp.PREV_DELAY_0)
                 .pass_through_delay(3))
        for i in range(3, 8):
            dp[i] = UopDpConfig().pass_through_alu().pass_through_delay(3)
        return u

    def s0(nx):            # park flopA = s + 0
        u = base(nx)
        u.datapath_config[3] = (UopDpConfig()
                                .enable_alu(AluOp.ADD, AluInp.PREV_ALU_OUT,
                                            AluInp.PREV_DELAY_3)
                                .pass_through_delay(3))
        return u

    def sa(nx):            # flopA += s
        u = base(nx)
        u.datapath_config[3] = (UopDpConfig()
                                .enable_alu(AluOp.ADD, AluInp.PREV_ALU_OUT,
                                            AluInp.CURR_ALU_OUT)
                                .pass_through_delay(3))
        return u

    def s4(nx):            # s3 off (s rides d1); flopB = s + 0
        u = base(nx)
        u.datapath_config[3] = (UopDpConfig()
                                .enable_delay_from_src(DelayInp.PREV_ALU_OUT, 1)
                                .pass_through_delay(3))
        u.datapath_config[4] = (UopDpConfig()
                                .enable_alu(AluOp.ADD, AluInp.PREV_DELAY_1,
                                            AluInp.PREV_DELAY_3)
                                .pass_through_delay(3))
        return u

    def sb(nx):            # flopB += s
        u = base(nx)
        u.datapath_config[3] = (UopDpConfig()
                                .enable_delay_from_src(DelayInp.PREV_ALU_OUT, 1)
                                .pass_through_delay(3))
        u.datapath_config[4] = (UopDpConfig()
                                .enable_alu(AluOp.ADD, AluInp.PREV_DELAY_1,
                                            AluInp.CURR_ALU_OUT)
                                .pass_through_delay(3))
        return u

    def s7(nx):            # emit q0 (flopA export via d2) + q1 (alu)
        u = base(nx)
        u.datapath_config[3] = (UopDpConfig()
                                .enable_alu(AluOp.ADD, AluInp.PREV_DELAY_3,
                                            AluInp.CURR_ALU_OUT)
                                .enable_delay_from_src(DelayInp.PREV_ALU_OUT, 1)
                                .pass_through_delay(3))
        u.datapath_config[4] = (UopDpConfig()
                                .enable_alu(AluOp.ADD, AluInp.PREV_DELAY_1,
                                            AluInp.CURR_ALU_OUT)
                                .enable_delay_from_src(DelayInp.PREV_ALU_OUT, 2)
                                .pass_through_delay(3))
        for i in range(5, 8):
            u.datapath_config[i] = (UopDpConfig().pass_through_alu()
                                    .pass_through_delay(2, 3))
        u.enable_output(OutSel.DELAY_2, OutPath.WR0_LO)
        u.enable_output(OutSel.ALU_OUT, OutPath.WR0_HI)
        return u

    # idx: 0=S0entry 1..7=S1..S7 8=S0loop (0 = IDLE, loop restarts at 8)
    return [s0(1), sa(2), sa(3), sa(4), s4(5), sb(6), sb(7), s7(8), s0(1)]


def _get_ps8_op():
    if "ps8" in _CACHED:
        return _CACHED["ps8"]
    from concourse import dve_ops
    from concourse.dve_spec import Spec, Src0, Src1, maxx
    from concourse.dve_uop import DveOpSpec

    NAME = "PAIRSUM8_ANT"
    for op in dve_ops.OPS:
        if op.name == NAME:
            _CACHED["ps8"] = op
            return op
    spec = Spec(
        body=maxx(Src0 - Src1, Src1 - Src0),
        reference=lambda in0, in1, s0, s1, imm2: np.abs(
            in0.astype(np.float32) - in1.astype(np.float32)),
    )
    op = dve_ops.DveOp(NAME, spec, subdim=False, uops_sha={})
    dve_ops.OPS.append(op)
    dve_ops.CUSTOM_DVE_SPECS[op.name] = op.spec
    row = dve_ops._CUSTOM_DVE_ROW_BASE + len(dve_ops.OPS) - 1
    dve_ops._SUB_OPCODE_FOR_NAME[op.name] = row
    uops = _mk_ps8_2x()
    compiled = DveOpSpec(
        name=NAME, opcode=row, uops=uops, uops_2x=uops,
        perf_max=1, rd1_en=True,
    )
    compiled.validate("v3")
    dve_ops._COMPILE_CACHE[(NAME, "v3")] = compiled
    dve_ops._COMPILE_CACHE[(NAME, "v4")] = compiled
    _CACHED["ps8"] = op
    return op


# --------------------------------------------------------------------------
# device program
# --------------------------------------------------------------------------

def make_pools(tc, ctx, rep=0):
    sfx = f"_{rep}"
    singles = ctx.enter_context(tc.tile_pool(name="singles" + sfx, bufs=1))
    xTp = ctx.enter_context(tc.tile_pool(name="xTp" + sfx, bufs=2))
    m2p = ctx.enter_context(tc.tile_pool(name="m2p" + sfx, bufs=2))
    psB = ctx.enter_context(tc.tile_pool(name="psB" + sfx, bufs=4,
                                         space="PSUM"))
    nrmp = ctx.enter_context(tc.tile_pool(name="nrmp" + sfx, bufs=2))
    Ep = ctx.enter_context(tc.tile_pool(name="Ep" + sfx, bufs=2))
    psO = ctx.enter_context(tc.tile_pool(name="psO" + sfx, bufs=2,
                                         space="PSUM"))
    osb = ctx.enter_context(tc.tile_pool(name="osb" + sfx, bufs=2))
    return singles, xTp, m2p, psB, nrmp, Ep, psO, osb


def build_body(tc, outs, ins, rep=0, pools=None):
    """Trace the per-core Tile program.

    ins:  xT  [128,2048] f16  xT[f,(fh,hh,i,hl)] = x[i, hw(core,hh,hl), fh*128+f]
          tw  [128,1024] f16  tw[f,(fh,c,b)] = T[fh*128+f, b, c]
          iw  [128,128]  f16  identity
    outs: o   [128,512]  f32  o[hh*64+b, i*16+hl] = o_b[i, hw(core,hh,hl), b]
    """
    from contextlib import ExitStack

    import concourse.mybir as mybir

    nc = tc.nc
    f16 = mybir.dt.float16
    f32 = mybir.dt.float32
    ps8 = _get_ps8_op()

    xT_d, tw_d, iw_d = ins["xT"], ins["tw"], ins["iw"]
    o_d = outs["o"]

    with ExitStack() as ctx:
        if pools is None:
            pools = make_pools(tc, ctx, rep)
        singles, xTp, m2p, psB, nrmp, Ep, psO, osb = pools

        # ---- loads
        tw_t = singles.tile([128, FH * C * B], f16, tag="tw")
        nc.sync.dma_start(out=tw_t, in_=tw_d)
        iw_t = singles.tile([128, 128], f16, tag="iw")
        nc.sync.dma_start(out=iw_t, in_=iw_d)
        xT_t = xTp.tile([128, FH * HH * N * HL], f16, tag="xT")
        nc.sync.dma_start(out=xT_t[:, 0:1024], in_=xT_d[:, 0:1024])
        nc.sync.dma_start(out=xT_t[:, 1024:2048], in_=xT_d[:, 1024:2048])
        tw_s = [[tw_t[:, (fh * C + c) * B:(fh * C + c + 1) * B]
                 for c in range(C)] for fh in range(FH)]
        xT_s = [[xT_t[:, (fh * HH + hh) * 512:(fh * HH + hh + 1) * 512]
                 for hh in range(HH)] for fh in range(FH)]

        # ---- stage B: M2[(hh,b), (i,hl,c)] = x @ T, one psum plane per c,
        # ACT-copied into the c-interleaved M2 layout.
        m2 = m2p.tile([128, IPAD * HL * C], f16, tag="m2")
        m2v = m2.rearrange("p (x c) -> p x c", c=C)
        for c in range(C):
            ps = psB.tile([128, 512], f32, tag="psB")
            for hh in range(HH):
                for fh in range(FH):
                    nc.tensor.matmul(
                        ps[64 * hh:64 * hh + 64, :],
                        lhsT=tw_s[fh][c], rhs=xT_s[fh][hh],
                        start=(fh == 0), stop=(fh == 1),
                        tile_position=(0, 64 * hh), skip_group_check=True,
                    )
            nc.scalar.copy(out=m2v[:, 0:512, c], in_=ps[:, :])
        # circular pad: i in [32,48) := i in [0,16)  (one contiguous block)
        nc.sync.dma_start(out=m2[:, 4096:6144], in_=m2[:, 0:2048])

        # ---- norms: ONE giant PAIRSUM8 instruction for d=1..15 + one for
        # d=16 (half range).  norm[d-1][i,hl] = sum_c |M2[i+d]-M2[i]|.
        nrm = nrmp.tile([128, 15 * 512 + 256], f16, tag="nrm")
        in0 = m2[:, 128:128 + 4096].unsqueeze(1).to_broadcast(
            [128, 15, 4096]).copy()
        # outer d dim: [step, num] = one i-step (128 elems) per d; base at d=1
        in0.ap[1] = (128, 15)
        in1 = m2[:, 0:4096].unsqueeze(1).to_broadcast([128, 15, 4096])
        out15 = nrm[:, 0:15 * 512].rearrange("p (d x) -> p d x", d=15)
        bi = nc.vector._custom_dve(ps8, out=out15, in0=in0, in1=in1)
        bi.ins.perf_max = 1
        bi = nc.vector._custom_dve(
            ps8, out=nrm[:, 7680:7936],
            in0=m2[:, 2048:4096], in1=m2[:, 0:2048])
        bi.ins.perf_max = 1

        # ---- E = exp(-nrm)  (4 ACT chunks)
        E = Ep.tile([128, 15 * 512 + 256], f16, tag="E")
        for a, b in ((0, 2048), (2048, 4096), (4096, 6144), (6144, 7936)):
            nc.scalar.activation(
                out=E[:, a:b], in_=nrm[:, a:b],
                func=mybir.ActivationFunctionType.Exp, scale=-1.0,
            )

        # ---- j-sum: identity matmuls accumulate exp terms into row i
        # (direct) and row i+d (shifted, column offset 16d mod 512).
        o_ps = psO.tile([128, 512], f32, tag="oPs")
        for d in range(1, 16):
            Ed = E[:, (d - 1) * 512:d * 512]
            sh = 16 * d
            nc.tensor.matmul(o_ps[:, :], lhsT=iw_t, rhs=Ed,
                             start=(d == 1), stop=False,
                             skip_group_check=True)
            nc.tensor.matmul(o_ps[:, sh:512], lhsT=iw_t,
                             rhs=Ed[:, 0:512 - sh],
                             start=False, stop=False, skip_group_check=True)
            nc.tensor.matmul(o_ps[:, 0:sh], lhsT=iw_t,
                             rhs=Ed[:, 512 - sh:512],
                             start=False, stop=False, skip_group_check=True)
        E16 = E[:, 7680:7936]
        nc.tensor.matmul(o_ps[:, 0:256], lhsT=iw_t, rhs=E16,
                         start=False, stop=False, skip_group_check=True)
        nc.tensor.matmul(o_ps[:, 256:512], lhsT=iw_t, rhs=E16,
                         start=False, stop=True, skip_group_check=True)

        # ---- diagonal (+1) fused into the PSUM drain, then DMA out
        o_sb = osb.tile([128, 512], f32, tag="osb")
        nc.scalar.activation(
            out=o_sb, in_=o_ps[:, :],
            func=mybir.ActivationFunctionType.Identity, bias=1.0, scale=1.0,
        )
        nc.sync.dma_start(out=o_d, in_=o_sb)


# --------------------------------------------------------------------------
# host side
# --------------------------------------------------------------------------

def prep_inputs(x, T):
    """Shared (core-independent) device inputs, packed partition-first."""
    xf = np.ascontiguousarray(x.reshape(N, HW, F))
    # tw[f, (fh, c, b)] = T[fh*128+f, b, c]
    tw = T.reshape(FH, 128, B, C).transpose(1, 0, 3, 2)     # f, fh, c, b
    tw_in = np.ascontiguousarray(tw.reshape(128, FH * C * B)).astype(np.float16)
    iw_in = np.eye(128, dtype=np.float16)
    return xf, tw_in, iw_in


def core_in_map(xf, tw_in, iw_in, k):
    xs = xf[:, k * HWL:(k + 1) * HWL, :]          # [i, hwl, f]
    # xT[f, (fh, hh, i, hl)] = x[i, hh*16+hl, fh*128+f]
    xT = xs.reshape(N, HH, HL, FH, 128).transpose(4, 3, 1, 0, 2)
    xT = np.ascontiguousarray(xT.reshape(128, FH * HH * N * HL))
    return {"xT": xT.astype(np.float16), "tw": tw_in, "iw": iw_in}


def gather_ob(core_outs):
    """core_outs: list of 8 arrays [128,512] f32 -> o_b [N,16,16,B]."""
    obs = []
    for res in core_outs:
        v = res.astype(np.float32).reshape(HH, B, N, HL)   # hh, b, i, hl
        obs.append(v.transpose(2, 0, 3, 1).reshape(N, HWL, B))  # i, hwl, b
    return np.concatenate(obs, axis=1).reshape(N, 16, 16, B)


def _get_program(reps=1, loop=None):
    key = ("nc", reps, loop)
    if key in _CACHED:
        return _CACHED[key]
    from contextlib import ExitStack
    import concourse.bacc as bacc
    import concourse.mybir as mybir
    import concourse.tile as tile

    _get_ps8_op()
    nc = bacc.Bacc("TRN2", target_bir_lowering=False, debug=False,
                   num_devices=CORES)
    f16, f32 = mybir.dt.float16, mybir.dt.float32
    ins = {
        "xT": nc.dram_tensor("xT", [128, FH * HH * N * HL], f16,
                             kind="ExternalInput").ap(),
        "tw": nc.dram_tensor("tw", [128, FH * C * B], f16,
                             kind="ExternalInput").ap(),
        "iw": nc.dram_tensor("iw", [128, 128], f16,
                             kind="ExternalInput").ap(),
    }
    outs = {
        "o": nc.dram_tensor("o", [128, 512], f32, kind="ExternalOutput").ap(),
    }
    with tile.TileContext(nc) as tc:
        if loop:
            with ExitStack() as ctx:
                pools = make_pools(tc, ctx)
                with tc.For_i(0, loop, 1,
                              hint_engines=(mybir.EngineType.PE,
                                            mybir.EngineType.DVE)):
                    build_body(tc, outs, ins, pools=pools)
        else:
            for r in range(reps):
                build_body(tc, outs, ins, rep=r)
    nc.compile()
    _CACHED[key] = nc
    return nc


def kernel(x, T):
    x = np.asarray(x, dtype=np.float32)
    T = np.asarray(T, dtype=np.float32)
    from concourse.bass_utils import run_bass_kernel_spmd

    nc = _get_program()
    xf, tw_in, iw_in = prep_inputs(x, T)
    in_maps = [core_in_map(xf, tw_in, iw_in, k) for k in range(CORES)]
    res = run_bass_kernel_spmd(nc, in_maps, core_ids=list(range(CORES)))
    ob = gather_ob([r["o"] for r in res.results])
    return np.concatenate([x, ob], axis=3)


# revision 7
# speedup vs baseline: 1.3446x; 1.3446x over previous
"""MiniBatchDiscrimination Trainium2 kernel — DVE-fused-norm version (v3).

reference:
    M = einsum('nhwf,fbc->nhwbc', x, T)          # [N,H,W,B,C]
    norm = sum_c |M[i] - M[j]|                   # [N,N,H,W,B]
    o_b  = sum_j exp(-norm)                      # [N,H,W,B]
    out  = concat([x, o_b], axis=3)              # [N,H,W,F+B]

Sharding: embarrassingly parallel over HW=256 spatial positions; each core
takes a 32-position hw slice and computes all pairs for it.

Layout (per core): the c-axis lives INNERMOST IN THE FREE DIM so a custom
8-state DVE micro-op ("PAIRSUM8", 2X_1PORT) computes, per (i,hwl,b) position,
the full L1 norm sum_c |M2[i+d] - M2[i]| in a single streaming pass —
consuming 2 f16/lane/cycle and emitting one compacted f16 norm per 8 inputs.
This removes the PE c-reduce matmuls entirely.

  M2   [128 part=(hh2,b64), 6144 free=(i48,hl16,c8)] f16; i in [32,48) is a
       circular pad (copy of i in [0,16)) so every d-stream is contiguous.
  nrm  [128, 7936 free=(d15,i32,hl16 | d16:i16,hl16)] f16 — ONE giant DVE
       instruction covers d=1..15 via a 3D AP (outer dim d, in1 broadcast).
  E    exp(-nrm) f16 (ACT)
  o_ps [128 part=(hh,b), 512=(i,hl)] f32 PSUM — j-sum via identity-weight
       matmuls: direct (col) + shifted (col+16d mod 512) accumulation.
  Pair symmetry: stream d covers pairs (i, (i+d)%32); exp(-norm) folds into
  row i (direct) and row i+d (shifted); diagonal +1 fused into the drain.
"""

import os
import sys

for _p in ("/opt/trn_rl_repo", "/opt/pypackages"):
    if _p not in sys.path and os.path.isdir(_p):
        sys.path.append(_p)

import numpy as np

N, HWL, F, B, C = 32, 32, 256, 64, 8
HW = 256
CORES = 8
FH = 2           # f in two partition halves of 128
HH = 2           # hwl_hi: hw position high bit (partition dim)
HL = 16          # hwl_lo: 16 positions (free dim)
IPAD = 48        # i padded 32 -> 48 for circular d-shift reads

F16 = "float16"

_CACHED = {}


# --------------------------------------------------------------------------
# PAIRSUM8 custom DVE op: out[g] = sum_{k<8} |in0[8g+k] - in1[8g+k]|
# 2X_1PORT 8-state machine; emits two group sums (WR0_LO/WR0_HI) every
# 4th pair-cycle; out FD = in FD / 8, contiguous.
# --------------------------------------------------------------------------

def _mk_ps8_2x():
    from concourse.dve_uop import (
        ENABLE, AluInp, AluOp, DelayInp, InpSel, OutPath, OutSel, Trigger,
        UopConfig, UopDpConfig,
    )

    def base(next_idx):
        u = UopConfig()
        u.enable_input(InpSel.SRC_0, 0).enable_input(InpSel.SRC_1, 1)
        u.enable_input(InpSel.SRC_0_HI, 2).enable_input(InpSel.SRC_1_HI, 3)
        u.enable_input(InpSel.ZERO, 4)
        u.require_inp0 = ENABLE
        u.require_inp1 = ENABLE
        u.trigger = (Trigger.SRC_TENSOR_DONE, Trigger.COUNT, Trigger.NONE)
        u.next_uop = (0, next_idx, 0)
        u.repeat_count = 1
        dp = u.datapath_config
        # s0: |a_lo-b_lo|; carry a_hi(d1), b_hi(d2), zero(d3)
        dp[0] = (UopDpConfig()
                 .enable_alu(AluOp.ABSOLUTE_DIFF, AluInp.PREV_ALU_OUT,
                             AluInp.PREV_DELAY_0)
                 .pass_through_delay(1, 2, 3))
        # s1: |a_hi-b_hi|; d0 <- lo result
        dp[1] = (UopDpConfig()
                 .enable_alu(AluOp.ABSOLUTE_DIFF, AluInp.PREV_DELAY_1,
                             AluInp.PREV_DELAY_2)
                 .enable_delay_from_src(DelayInp.PREV_ALU_OUT, 0)
                 .pass_through_delay(3))
        # s2: pair sum s = lo + hi
        dp[2] = (UopDpConfig()
                 .enable_alu(AluOp.ADD, AluInp.PREV_ALU_OUT,
                             AluInp.PREV_DELAY_0)
                 .pass_through_delay(3))
        for i in range(3, 8):
            dp[i] = UopDpConfig().pass_through_alu().pass_through_delay(3)
        return u

    def s0(nx):            # park flopA = s + 0
        u = base(nx)
        u.datapath_config[3] = (UopDpConfig()
                                .enable_alu(AluOp.ADD, AluInp.PREV_ALU_OUT,
                                            AluInp.PREV_DELAY_3)
                                .pass_through_delay(3))
        return u

    def sa(nx):            # flopA += s
        u = base(nx)
        u.datapath_config[3] = (UopDpConfig()
                                .enable_alu(AluOp.ADD, AluInp.PREV_ALU_OUT,
                                            AluInp.CURR_ALU_OUT)
                                .pass_through_delay(3))
        return u

    def s4(nx):            # s3 off (s rides d1); flopB = s + 0
        u = base(nx)
        u.datapath_config[3] = (UopDpConfig()
                                .enable_delay_from_src(DelayInp.PREV_ALU_OUT, 1)
                                .pass_through_delay(3))
        u.datapath_config[4] = (UopDpConfig()
                                .enable_alu(AluOp.ADD, AluInp.PREV_DELAY_1,
                                            AluInp.PREV_DELAY_3)
                                .pass_through_delay(3))
        return u

    def sb(nx):            # flopB += s
        u = base(nx)
        u.datapath_config[3] = (UopDpConfig()
                                .enable_delay_from_src(DelayInp.PREV_ALU_OUT, 1)
                                .pass_through_delay(3))
        u.datapath_config[4] = (UopDpConfig()
                                .enable_alu(AluOp.ADD, AluInp.PREV_DELAY_1,
                                            AluInp.CURR_ALU_OUT)
                                .pass_through_delay(3))
        return u

    def s7(nx):            # emit q0 (flopA export via d2) + q1 (alu)
        u = base(nx)
        u.datapath_config[3] = (UopDpConfig()
                                .enable_alu(AluOp.ADD, AluInp.PREV_DELAY_3,
                                            AluInp.CURR_ALU_OUT)
                                .enable_delay_from_src(DelayInp.PREV_ALU_OUT, 1)
                                .pass_through_delay(3))
        u.datapath_config[4] = (UopDpConfig()
                                .enable_alu(AluOp.ADD, AluInp.PREV_DELAY_1,
                                            AluInp.CURR_ALU_OUT)
                                .enable_delay_from_src(DelayInp.PREV_ALU_OUT, 2)
                                .pass_through_delay(3))
        for i in range(5, 8):
            u.datapath_config[i] = (UopDpConfig().pass_through_alu()
                                    .pass_through_delay(2, 3))
        u.enable_output(OutSel.DELAY_2, OutPath.WR0_LO)
        u.enable_output(OutSel.ALU_OUT, OutPath.WR0_HI)
        return u

    # idx: 0=S0entry 1..7=S1..S7 8=S0loop (0 = IDLE, loop restarts at 8)
    return [s0(1), sa(2), sa(3), sa(4), s4(5), sb(6), sb(7), s7(8), s0(1)]


def _get_ps8_op():
    if "ps8" in _CACHED:
        return _CACHED["ps8"]
    from concourse import dve_ops
    from concourse.dve_spec import Spec, Src0, Src1, maxx
    from concourse.dve_uop import DveOpSpec

    NAME = "PAIRSUM8_ANT"
    for op in dve_ops.OPS:
        if op.name == NAME:
            _CACHED["ps8"] = op
            return op
    spec = Spec(
        body=maxx(Src0 - Src1, Src1 - Src0),
        reference=lambda in0, in1, s0, s1, imm2: np.abs(
            in0.astype(np.float32) - in1.astype(np.float32)),
    )
    op = dve_ops.DveOp(NAME, spec, subdim=False, uops_sha={})
    dve_ops.OPS.append(op)
    dve_ops.CUSTOM_DVE_SPECS[op.name] = op.spec
    row = dve_ops._CUSTOM_DVE_ROW_BASE + len(dve_ops.OPS) - 1
    dve_ops._SUB_OPCODE_FOR_NAME[op.name] = row
    uops = _mk_ps8_2x()
    compiled = DveOpSpec(
        name=NAME, opcode=row, uops=uops, uops_2x=uops,
        perf_max=1, rd1_en=True,
    )
    compiled.validate("v3")
    dve_ops._COMPILE_CACHE[(NAME, "v3")] = compiled
    dve_ops._COMPILE_CACHE[(NAME, "v4")] = compiled
    _CACHED["ps8"] = op
    return op


# --------------------------------------------------------------------------
# device program
# --------------------------------------------------------------------------

def make_pools(tc, ctx, rep=0):
    sfx = f"_{rep}"
    singles = ctx.enter_context(tc.tile_pool(name="singles" + sfx, bufs=1))
    xTp = ctx.enter_context(tc.tile_pool(name="xTp" + sfx, bufs=2))
    m2p = ctx.enter_context(tc.tile_pool(name="m2p" + sfx, bufs=2))
    psB = ctx.enter_context(tc.tile_pool(name="psB" + sfx, bufs=3,
                                         space="PSUM"))
    nrmp = ctx.enter_context(tc.tile_pool(name="nrmp" + sfx, bufs=2))
    Ep = ctx.enter_context(tc.tile_pool(name="Ep" + sfx, bufs=2))
    psO = ctx.enter_context(tc.tile_pool(name="psO" + sfx, bufs=2,
                                         space="PSUM"))
    osb = ctx.enter_context(tc.tile_pool(name="osb" + sfx, bufs=2))
    return singles, xTp, m2p, psB, nrmp, Ep, psO, osb


def build_body(tc, outs, ins, rep=0, pools=None):
    """Trace the per-core Tile program.

    ins:  xT  [128,2048] f16  xT[f,(fh,hh,i,hl)] = x[i, hw(core,hh,hl), fh*128+f]
          tw  [128,1024] f16  tw[f,(fh,c,b)] = T[fh*128+f, b, c]
          iw  [128,128]  f16  identity
    outs: o   [128,512]  f32  o[hh*64+b, i*16+hl] = o_b[i, hw(core,hh,hl), b]
    """
    from contextlib import ExitStack

    import concourse.mybir as mybir

    nc = tc.nc
    f16 = mybir.dt.float16
    f32 = mybir.dt.float32
    ps8 = _get_ps8_op()

    xT_d, tw_d, iw_d = ins["xT"], ins["tw"], ins["iw"]
    o_d = outs["o"]

    with ExitStack() as ctx:
        if pools is None:
            pools = make_pools(tc, ctx, rep)
        singles, xTp, m2p, psB, nrmp, Ep, psO, osb = pools

        # ---- loads
        tw_t = singles.tile([128, FH * C * B], f16, tag="tw")
        nc.sync.dma_start(out=tw_t, in_=tw_d)
        iw_t = singles.tile([128, 128], f16, tag="iw")
        nc.sync.dma_start(out=iw_t, in_=iw_d)
        xT_t = xTp.tile([128, FH * HH * N * HL], f16, tag="xT")
        nc.sync.dma_start(out=xT_t[:, 0:1024], in_=xT_d[:, 0:1024])
        nc.sync.dma_start(out=xT_t[:, 1024:2048], in_=xT_d[:, 1024:2048])
        tw_s = [[tw_t[:, (fh * C + c) * B:(fh * C + c + 1) * B]
                 for c in range(C)] for fh in range(FH)]
        xT_s = [[xT_t[:, (fh * HH + hh) * 512:(fh * HH + hh + 1) * 512]
                 for hh in range(HH)] for fh in range(FH)]

        # ---- stage B: M2[(hh,b), (i,hl,c)] = x @ T.  Two psum groups of 4
        # c-planes each; one grouped (transposing) ACT copy per psum tile
        # interleaves c into the M2 layout.
        m2 = m2p.tile([128, IPAD * HL * C], f16, tag="m2")
        m2v = m2.rearrange("p (x c) -> p x c", c=C)
        for g in range(4):
            ps = psB.tile([128, 1024], f32, tag="psB")
            for cl in range(2):
                c = 2 * g + cl
                for hh in range(HH):
                    for fh in range(FH):
                        nc.tensor.matmul(
                            ps[64 * hh:64 * hh + 64, cl * 512:(cl + 1) * 512],
                            lhsT=tw_s[fh][c], rhs=xT_s[fh][hh],
                            start=(fh == 0), stop=(fh == 1),
                            tile_position=(0, 64 * hh), skip_group_check=True,
                        )
            # out (x, c) interleave <- in (c, x) planes
            out_ap = m2v[:, 0:512, 2 * g:2 * g + 2]
            in_ap = ps.rearrange("p (c x) -> p x c", c=2)
            nc.scalar.copy(out=out_ap, in_=in_ap)
        # circular pad: i in [32,48) := i in [0,16)  (DVE is idle here; a
        # single 4x-mode copy is ~0.6us)
        nc.vector.tensor_copy(out=m2[:, 4096:6144], in_=m2[:, 0:2048])

        # ---- chunked norm -> exp -> j-sum pipeline over d-streams.
        # norm[d-1][i,hl] = sum_c |M2[i+d]-M2[i]|; chunk k's exp/j-sum
        # overlaps chunk k+1's DVE pass.
        CHUNKS = ((1, 5), (6, 10), (11, 14), (15, 16))
        nrm = nrmp.tile([128, 15 * 512 + 256], f16, tag="nrm")
        E = Ep.tile([128, 15 * 512 + 256], f16, tag="E")
        o_ps = psO.tile([128, 512], f32, tag="oPs")

        def dve_chunk(d0, d1):
            nd_full = min(d1, 15) - d0 + 1
            if nd_full > 0:
                in0 = m2[:, 128 * d0:128 * d0 + 4096].unsqueeze(
                    1).to_broadcast([128, nd_full, 4096]).copy()
                in0.ap[1] = (128, nd_full)    # [step, num]: one i per d
                in1 = m2[:, 0:4096].unsqueeze(1).to_broadcast(
                    [128, nd_full, 4096])
                o = nrm[:, (d0 - 1) * 512:(d0 - 1 + nd_full) * 512]
                bi = nc.vector._custom_dve(
                    ps8, out=o.rearrange("p (d x) -> p d x", d=nd_full),
                    in0=in0, in1=in1)
                bi.ins.perf_max = 1
            if d1 == 16:
                bi = nc.vector._custom_dve(
                    ps8, out=nrm[:, 7680:7936],
                    in0=m2[:, 2048:4096], in1=m2[:, 0:2048])
                bi.ins.perf_max = 1

        def exp_chunk(d0, d1):
            a = (d0 - 1) * 512
            b = min(d1, 15) * 512 if d1 < 16 else 7936
            nc.scalar.activation(
                out=E[:, a:b], in_=nrm[:, a:b],
                func=mybir.ActivationFunctionType.Exp, scale=-1.0,
            )

        def jsum_chunk(d0, d1):
            for d in range(d0, d1 + 1):
                if d < 16:
                    Ed = E[:, (d - 1) * 512:d * 512]
                    sh = 16 * d
                    nc.tensor.matmul(o_ps[:, :], lhsT=iw_t, rhs=Ed,
                                     start=(d == 1), stop=False,
                                     skip_group_check=True)
                    nc.tensor.matmul(o_ps[:, sh:512], lhsT=iw_t,
                                     rhs=Ed[:, 0:512 - sh], start=False,
                                     stop=False, skip_group_check=True)
                    nc.tensor.matmul(o_ps[:, 0:sh], lhsT=iw_t,
                                     rhs=Ed[:, 512 - sh:512], start=False,
                                     stop=False, skip_group_check=True)
                else:
                    E16 = E[:, 7680:7936]
                    nc.tensor.matmul(o_ps[:, 0:256], lhsT=iw_t, rhs=E16,
                                     start=False, stop=False,
                                     skip_group_check=True)
                    nc.tensor.matmul(o_ps[:, 256:512], lhsT=iw_t, rhs=E16,
                                     start=False, stop=True,
                                     skip_group_check=True)

        for ci, (d0, d1) in enumerate(CHUNKS):
            dve_chunk(d0, d1)
            if ci > 0:
                exp_chunk(*CHUNKS[ci - 1])
                jsum_chunk(*CHUNKS[ci - 1])
        exp_chunk(*CHUNKS[-1])
        jsum_chunk(*CHUNKS[-1])

        # ---- diagonal (+1) fused into the PSUM drain, then DMA out
        o_sb = osb.tile([128, 512], f32, tag="osb")
        nc.scalar.activation(
            out=o_sb, in_=o_ps[:, :],
            func=mybir.ActivationFunctionType.Identity, bias=1.0, scale=1.0,
        )
        nc.sync.dma_start(out=o_d, in_=o_sb)


# --------------------------------------------------------------------------
# host side
# --------------------------------------------------------------------------

def prep_inputs(x, T):
    """Shared (core-independent) device inputs, packed partition-first."""
    xf = np.ascontiguousarray(x.reshape(N, HW, F))
    # tw[f, (fh, c, b)] = T[fh*128+f, b, c]
    tw = T.reshape(FH, 128, B, C).transpose(1, 0, 3, 2)     # f, fh, c, b
    tw_in = np.ascontiguousarray(tw.reshape(128, FH * C * B)).astype(np.float16)
    iw_in = np.eye(128, dtype=np.float16)
    return xf, tw_in, iw_in


def core_in_map(xf, tw_in, iw_in, k):
    xs = xf[:, k * HWL:(k + 1) * HWL, :]          # [i, hwl, f]
    # xT[f, (fh, hh, i, hl)] = x[i, hh*16+hl, fh*128+f]
    xT = xs.reshape(N, HH, HL, FH, 128).transpose(4, 3, 1, 0, 2)
    xT = np.ascontiguousarray(xT.reshape(128, FH * HH * N * HL))
    return {"xT": xT.astype(np.float16), "tw": tw_in, "iw": iw_in}


def gather_ob(core_outs):
    """core_outs: list of 8 arrays [128,512] f32 -> o_b [N,16,16,B]."""
    obs = []
    for res in core_outs:
        v = res.astype(np.float32).reshape(HH, B, N, HL)   # hh, b, i, hl
        obs.append(v.transpose(2, 0, 3, 1).reshape(N, HWL, B))  # i, hwl, b
    return np.concatenate(obs, axis=1).reshape(N, 16, 16, B)


def _get_program(reps=1, loop=None):
    key = ("nc", reps, loop)
    if key in _CACHED:
        return _CACHED[key]
    from contextlib import ExitStack
    import concourse.bacc as bacc
    import concourse.mybir as mybir
    import concourse.tile as tile

    _get_ps8_op()
    nc = bacc.Bacc("TRN2", target_bir_lowering=False, debug=False,
                   num_devices=CORES)
    f16, f32 = mybir.dt.float16, mybir.dt.float32
    ins = {
        "xT": nc.dram_tensor("xT", [128, FH * HH * N * HL], f16,
                             kind="ExternalInput").ap(),
        "tw": nc.dram_tensor("tw", [128, FH * C * B], f16,
                             kind="ExternalInput").ap(),
        "iw": nc.dram_tensor("iw", [128, 128], f16,
                             kind="ExternalInput").ap(),
    }
    outs = {
        "o": nc.dram_tensor("o", [128, 512], f32, kind="ExternalOutput").ap(),
    }
    with tile.TileContext(nc) as tc:
        if loop:
            with ExitStack() as ctx:
                pools = make_pools(tc, ctx)
                with tc.For_i(0, loop, 1,
                              hint_engines=(mybir.EngineType.PE,
                                            mybir.EngineType.DVE)):
                    build_body(tc, outs, ins, pools=pools)
        else:
            for r in range(reps):
                build_body(tc, outs, ins, rep=r)
    nc.compile()
    _CACHED[key] = nc
    return nc


def kernel(x, T):
    x = np.asarray(x, dtype=np.float32)
    T = np.asarray(T, dtype=np.float32)
    from concourse.bass_utils import run_bass_kernel_spmd

    nc = _get_program()
    xf, tw_in, iw_in = prep_inputs(x, T)
    in_maps = [core_in_map(xf, tw_in, iw_in, k) for k in range(CORES)]
    res = run_bass_kernel_spmd(nc, in_maps, core_ids=list(range(CORES)))
    ob = gather_ob([r["o"] for r in res.results])
    return np.concatenate([x, ob], axis=3)


# revision 16
# speedup vs baseline: 1.4755x; 1.0974x over previous
"""MiniBatchDiscrimination Trainium2 kernel — DVE-fused-norm version (v3).

reference:
    M = einsum('nhwf,fbc->nhwbc', x, T)          # [N,H,W,B,C]
    norm = sum_c |M[i] - M[j]|                   # [N,N,H,W,B]
    o_b  = sum_j exp(-norm)                      # [N,H,W,B]
    out  = concat([x, o_b], axis=3)              # [N,H,W,F+B]

Sharding: embarrassingly parallel over HW=256 spatial positions; each core
takes a 32-position hw slice and computes all pairs for it.

Layout (per core): the c-axis lives INNERMOST IN THE FREE DIM so a custom
8-state DVE micro-op ("PAIRSUM8", 2X_1PORT) computes, per (i,hwl,b) position,
the full L1 norm sum_c |M2[i+d] - M2[i]| in a single streaming pass —
consuming 2 f16/lane/cycle and emitting one compacted f16 norm per 8 inputs.
This removes the PE c-reduce matmuls entirely.

  M2   [128 part=(hh2,b64), 6144 free=(i48,hl16,c8)] f16; i in [32,48) is a
       circular pad (copy of i in [0,16)) so every d-stream is contiguous.
  nrm  [128, 7936 free=(d15,i32,hl16 | d16:i16,hl16)] f16 — ONE giant DVE
       instruction covers d=1..15 via a 3D AP (outer dim d, in1 broadcast).
  E    exp(-nrm) f16 (ACT)
  o_ps [128 part=(hh,b), 512=(i,hl)] f32 PSUM — j-sum via identity-weight
       matmuls: direct (col) + shifted (col+16d mod 512) accumulation.
  Pair symmetry: stream d covers pairs (i, (i+d)%32); exp(-norm) folds into
  row i (direct) and row i+d (shifted); diagonal +1 fused into the drain.
"""

import os
import sys

for _p in ("/opt/trn_rl_repo", "/opt/pypackages"):
    if _p not in sys.path and os.path.isdir(_p):
        sys.path.append(_p)

import numpy as np

N, HWL, F, B, C = 32, 32, 256, 64, 8
HW = 256
CORES = 8
FH = 2           # f in two partition halves of 128
HH = 2           # hwl_hi: hw position high bit (partition dim)
HL = 16          # hwl_lo: 16 positions (free dim)
IPAD = 48        # i padded 32 -> 48 for circular d-shift reads

F16 = "float16"

_CACHED = {}


# --------------------------------------------------------------------------
# PAIRSUM8 custom DVE op: out[g] = sum_{k<8} |in0[8g+k] - in1[8g+k]|
# 2X_1PORT 8-state machine; emits two group sums (WR0_LO/WR0_HI) every
# 4th pair-cycle; out FD = in FD / 8, contiguous.
# --------------------------------------------------------------------------

def _mk_ps8_2x():
    from concourse.dve_uop import (
        ENABLE, AluInp, AluOp, DelayInp, InpSel, OutPath, OutSel, Trigger,
        UopConfig, UopDpConfig,
    )

    def base(next_idx):
        u = UopConfig()
        u.enable_input(InpSel.SRC_0, 0).enable_input(InpSel.SRC_1, 1)
        u.enable_input(InpSel.SRC_0_HI, 2).enable_input(InpSel.SRC_1_HI, 3)
        u.enable_input(InpSel.ZERO, 4)
        u.require_inp0 = ENABLE
        u.require_inp1 = ENABLE
        u.trigger = (Trigger.SRC_TENSOR_DONE, Trigger.COUNT, Trigger.NONE)
        u.next_uop = (0, next_idx, 0)
        u.repeat_count = 1
        dp = u.datapath_config
        # s0: |a_lo-b_lo|; carry a_hi(d1), b_hi(d2), zero(d3)
        dp[0] = (UopDpConfig()
                 .enable_alu(AluOp.ABSOLUTE_DIFF, AluInp.PREV_ALU_OUT,
                             AluInp.PREV_DELAY_0)
                 .pass_through_delay(1, 2, 3))
        # s1: |a_hi-b_hi|; d0 <- lo result
        dp[1] = (UopDpConfig()
                 .enable_alu(AluOp.ABSOLUTE_DIFF, AluInp.PREV_DELAY_1,
                             AluInp.PREV_DELAY_2)
                 .enable_delay_from_src(DelayInp.PREV_ALU_OUT, 0)
                 .pass_through_delay(3))
        # s2: pair sum s = lo + hi
        dp[2] = (UopDpConfig()
                 .enable_alu(AluOp.ADD, AluInp.PREV_ALU_OUT,
                             AluInp.PREV_DELAY_0)
                 .pass_through_delay(3))
        for i in range(3, 8):
            dp[i] = UopDpConfig().pass_through_alu().pass_through_delay(3)
        return u

    def s0(nx):            # park flopA = s + 0
        u = base(nx)
        u.datapath_config[3] = (UopDpConfig()
                                .enable_alu(AluOp.ADD, AluInp.PREV_ALU_OUT,
                                            AluInp.PREV_DELAY_3)
                                .pass_through_delay(3))
        return u

    def sa(nx):            # flopA += s
        u = base(nx)
        u.datapath_config[3] = (UopDpConfig()
                                .enable_alu(AluOp.ADD, AluInp.PREV_ALU_OUT,
                                            AluInp.CURR_ALU_OUT)
                                .pass_through_delay(3))
        return u

    def s4(nx):            # s3 off (s rides d1); flopB = s + 0
        u = base(nx)
        u.datapath_config[3] = (UopDpConfig()
                                .enable_delay_from_src(DelayInp.PREV_ALU_OUT, 1)
                                .pass_through_delay(3))
        u.datapath_config[4] = (UopDpConfig()
                                .enable_alu(AluOp.ADD, AluInp.PREV_DELAY_1,
                                            AluInp.PREV_DELAY_3)
                                .pass_through_delay(3))
        return u

    def sb(nx):            # flopB += s
        u = base(nx)
        u.datapath_config[3] = (UopDpConfig()
                                .enable_delay_from_src(DelayInp.PREV_ALU_OUT, 1)
                                .pass_through_delay(3))
        u.datapath_config[4] = (UopDpConfig()
                                .enable_alu(AluOp.ADD, AluInp.PREV_DELAY_1,
                                            AluInp.CURR_ALU_OUT)
                                .pass_through_delay(3))
        return u

    def s7(nx):            # emit q0 (flopA export via d2) + q1 (alu)
        u = base(nx)
        u.datapath_config[3] = (UopDpConfig()
                                .enable_alu(AluOp.ADD, AluInp.PREV_DELAY_3,
                                            AluInp.CURR_ALU_OUT)
                                .enable_delay_from_src(DelayInp.PREV_ALU_OUT, 1)
                                .pass_through_delay(3))
        u.datapath_config[4] = (UopDpConfig()
                                .enable_alu(AluOp.ADD, AluInp.PREV_DELAY_1,
                                            AluInp.CURR_ALU_OUT)
                                .enable_delay_from_src(DelayInp.PREV_ALU_OUT, 2)
                                .pass_through_delay(3))
        for i in range(5, 8):
            u.datapath_config[i] = (UopDpConfig().pass_through_alu()
                                    .pass_through_delay(2, 3))
        u.enable_output(OutSel.DELAY_2, OutPath.WR0_LO)
        u.enable_output(OutSel.ALU_OUT, OutPath.WR0_HI)
        return u

    # idx: 0=S0entry 1..7=S1..S7 8=S0loop (0 = IDLE, loop restarts at 8)
    return [s0(1), sa(2), sa(3), sa(4), s4(5), sb(6), sb(7), s7(8), s0(1)]


def _get_ps8_op():
    if "ps8" in _CACHED:
        return _CACHED["ps8"]
    from concourse import dve_ops
    from concourse.dve_spec import Spec, Src0, Src1, maxx
    from concourse.dve_uop import DveOpSpec

    NAME = "PAIRSUM8_ANT"
    for op in dve_ops.OPS:
        if op.name == NAME:
            _CACHED["ps8"] = op
            return op
    spec = Spec(
        body=maxx(Src0 - Src1, Src1 - Src0),
        reference=lambda in0, in1, s0, s1, imm2: np.abs(
            in0.astype(np.float32) - in1.astype(np.float32)),
    )
    op = dve_ops.DveOp(NAME, spec, subdim=False, uops_sha={})
    dve_ops.OPS.append(op)
    dve_ops.CUSTOM_DVE_SPECS[op.name] = op.spec
    row = dve_ops._CUSTOM_DVE_ROW_BASE + len(dve_ops.OPS) - 1
    dve_ops._SUB_OPCODE_FOR_NAME[op.name] = row
    uops = _mk_ps8_2x()
    compiled = DveOpSpec(
        name=NAME, opcode=row, uops=uops, uops_2x=uops,
        perf_max=1, rd1_en=True,
    )
    compiled.validate("v3")
    dve_ops._COMPILE_CACHE[(NAME, "v3")] = compiled
    dve_ops._COMPILE_CACHE[(NAME, "v4")] = compiled
    _CACHED["ps8"] = op
    return op


# --------------------------------------------------------------------------
# device program
# --------------------------------------------------------------------------

def make_pools(tc, ctx, rep=0):
    sfx = f"_{rep}"
    singles = ctx.enter_context(tc.tile_pool(name="singles" + sfx, bufs=1))
    xTp = ctx.enter_context(tc.tile_pool(name="xTp" + sfx, bufs=2))
    m2p = ctx.enter_context(tc.tile_pool(name="m2p" + sfx, bufs=2))
    psB = ctx.enter_context(tc.tile_pool(name="psB" + sfx, bufs=3,
                                         space="PSUM"))
    nrmp = ctx.enter_context(tc.tile_pool(name="nrmp" + sfx, bufs=2))
    Ep = ctx.enter_context(tc.tile_pool(name="Ep" + sfx, bufs=2))
    psO = ctx.enter_context(tc.tile_pool(name="psO" + sfx, bufs=2,
                                         space="PSUM"))
    osb = ctx.enter_context(tc.tile_pool(name="osb" + sfx, bufs=2))
    return singles, xTp, m2p, psB, nrmp, Ep, psO, osb


def build_body(tc, outs, ins, rep=0, pools=None, parts="all"):
    """Trace the per-core Tile program.

    ins:  xT  [128,2048] f16  xT[f,(fh,hh,i,hl)] = x[i, hw(core,hh,hl), fh*128+f]
          tw  [128,1024] f16  tw[f,(fh,c,b)] = T[fh*128+f, b, c]
          iw  [128,128]  f16  identity
    outs: o   [128,512]  f32  o[hh*64+b, i*16+hl] = o_b[i, hw(core,hh,hl), b]
    """
    from contextlib import ExitStack

    import concourse.mybir as mybir

    nc = tc.nc
    f16 = mybir.dt.float16
    f32 = mybir.dt.float32
    ps8 = _get_ps8_op()

    xT_d, tw_d, iw_d = ins["xT"], ins["tw"], ins["iw"]
    o_d = outs["o"]

    with ExitStack() as ctx:
        if pools is None:
            pools = make_pools(tc, ctx, rep)
        singles, xTp, m2p, psB, nrmp, Ep, psO, osb = pools

        # ---- loads
        tw_t = singles.tile([128, FH * C * B], f16, tag="tw")
        nc.sync.dma_start(out=tw_t, in_=tw_d)
        iw_t = singles.tile([128, 128], f16, tag="iw")
        nc.sync.dma_start(out=iw_t, in_=iw_d)
        xT_t = xTp.tile([128, FH * HH * N * HL], f16, tag="xT")
        nc.sync.dma_start(out=xT_t[:, 0:1024], in_=xT_d[:, 0:1024])
        nc.sync.dma_start(out=xT_t[:, 1024:2048], in_=xT_d[:, 1024:2048])
        tw_s = [[tw_t[:, (fh * C + c) * B:(fh * C + c + 1) * B]
                 for c in range(C)] for fh in range(FH)]
        xT_s = [[xT_t[:, (fh * HH + hh) * 512:(fh * HH + hh + 1) * 512]
                 for hh in range(HH)] for fh in range(FH)]

        # ---- per-hl-half pipeline.  The per-core problem splits into two
        # independent halves over hl (hw positions (hh, hlh, hl8)): half B's
        # stage B runs on PE/ACT while half A streams on the DVE, so only
        # the first half's stage B is latency-exposed.
        # M2 free layout: (hlh2, i48, hl8, c8); xT cols: (fh, hh, hlh, i, hl8)
        m2 = m2p.tile([128, 2 * IPAD * 8 * C], f16, tag="m2")
        nrm = nrmp.tile([128, 2 * (15 * 256 + 128)], f16, tag="nrm")
        E = Ep.tile([128, 2 * (15 * 256 + 128)], f16, tag="E")
        o_ps = psO.tile([128, 512], f32, tag="oPs")
        CHUNKS = ((1, 5), (6, 10), (11, 15), (16, 16))
        NB = 15 * 256 + 128          # nrm/E block size per half

        def stage_b(hlh):
            # 8 matmuls (2 psum groups of 4 c-planes), grouped transposing
            # ACT copies into the c-interleaved M2 half.
            for g in range(2):
                ps = psB.tile([128, 1024], f32, tag="psB")
                for cl in range(4):
                    c = 4 * g + cl
                    for hh in range(HH):
                        for fh in range(FH):
                            nc.tensor.matmul(
                                ps[64 * hh:64 * hh + 64,
                                   cl * 256:(cl + 1) * 256],
                                lhsT=tw_s[fh][c],
                                rhs=xT_s[fh][hh][:, hlh * 256:(hlh + 1) * 256],
                                start=(fh == 0), stop=(fh == 1),
                                tile_position=(0, 64 * hh),
                                skip_group_check=True,
                            )
                # m2[hlh*3072 + x*8 + c] <- ps[(c4, x256)]
                out_ap = m2[:, hlh * 3072:hlh * 3072 + 2048].rearrange(
                    "p (x c) -> p x c", c=C)[:, :, 4 * g:4 * g + 4]
                in_ap = ps.rearrange("p (c x) -> p x c", c=4)
                nc.scalar.copy(out=out_ap, in_=in_ap)
            # circular pad: i in [32,48) := i in [0,16)
            nc.vector.tensor_copy(
                out=m2[:, hlh * 3072 + 2048:hlh * 3072 + 3072],
                in_=m2[:, hlh * 3072:hlh * 3072 + 1024])

        def dve_chunk(hlh, d0, d1):
            base = hlh * 3072
            nd_full = min(d1, 15) - d0 + 1
            if nd_full > 0:
                in0 = m2[:, base + 64 * d0:base + 64 * d0 + 2048].unsqueeze(
                    1).to_broadcast([128, nd_full, 2048]).copy()
                in0.ap[1] = (64, nd_full)    # [step, num]: one i per d
                in1 = m2[:, base:base + 2048].unsqueeze(1).to_broadcast(
                    [128, nd_full, 2048])
                o = nrm[:, hlh * NB + (d0 - 1) * 256:
                        hlh * NB + (d0 - 1 + nd_full) * 256]
                bi = nc.vector._custom_dve(
                    ps8, out=o.rearrange("p (d x) -> p d x", d=nd_full),
                    in0=in0, in1=in1)
                bi.ins.perf_max = 1
            if d1 == 16:
                bi = nc.vector._custom_dve(
                    ps8, out=nrm[:, hlh * NB + 3840:hlh * NB + 3968],
                    in0=m2[:, base + 1024:base + 2048],
                    in1=m2[:, base:base + 1024])
                bi.ins.perf_max = 1

        def exp_chunk(hlh, d0, d1):
            a = hlh * NB + (d0 - 1) * 256
            b = hlh * NB + (min(d1, 15) * 256 if d1 < 16 else 3968)
            nc.scalar.activation(
                out=E[:, a:b], in_=nrm[:, a:b],
                func=mybir.ActivationFunctionType.Exp, scale=-1.0,
            )

        def jsum_chunk(hlh, d0, d1, first, last):
            ob = hlh * 256            # o_ps column block for this half
            for d in range(d0, d1 + 1):
                if d < 16:
                    Ed = E[:, hlh * NB + (d - 1) * 256:hlh * NB + d * 256]
                    sh = 8 * d
                    nc.tensor.matmul(o_ps[:, ob:ob + 256], lhsT=iw_t, rhs=Ed,
                                     start=(first and d == d0), stop=False,
                                     skip_group_check=True)
                    nc.tensor.matmul(o_ps[:, ob + sh:ob + 256], lhsT=iw_t,
                                     rhs=Ed[:, 0:256 - sh], start=False,
                                     stop=False, skip_group_check=True)
                    nc.tensor.matmul(o_ps[:, ob:ob + sh], lhsT=iw_t,
                                     rhs=Ed[:, 256 - sh:256], start=False,
                                     stop=False, skip_group_check=True)
                else:
                    E16 = E[:, hlh * NB + 3840:hlh * NB + 3968]
                    nc.tensor.matmul(o_ps[:, ob:ob + 128], lhsT=iw_t, rhs=E16,
                                     start=False, stop=False,
                                     skip_group_check=True)
                    nc.tensor.matmul(o_ps[:, ob + 128:ob + 256], lhsT=iw_t,
                                     rhs=E16, start=False, stop=last,
                                     skip_group_check=True)

        do_dve = parts in ("all", "dve", "exp")
        do_exp = parts in ("all", "exp")
        do_jsum = parts == "all"
        # software pipeline: [sb(0), dve(0,c0), sb(1), dve(0,c1) || exp/jsum
        # trailing one chunk behind, dve(1,*) ...]
        stage_b(0)
        prev = None
        for hlh in range(2):
            if hlh == 1 and True:
                pass
            for ci, (d0, d1) in enumerate(CHUNKS):
                if do_dve:
                    dve_chunk(hlh, d0, d1)
                if hlh == 0 and ci == 0:
                    stage_b(1)
                if prev is not None:
                    ph, pd0, pd1 = prev
                    if do_exp:
                        exp_chunk(ph, pd0, pd1)
                    if do_jsum:
                        jsum_chunk(ph, pd0, pd1,
                                   first=(ph == 0 and pd0 == 1), last=False)
                prev = (hlh, d0, d1)
        if do_exp:
            exp_chunk(*prev)
        if do_jsum:
            jsum_chunk(*prev, first=False, last=True)
        if not do_jsum:
            nc.vector.memset(o_ps[:, :], 0.0)
            if not do_exp:
                nc.vector.memset(E[:, 0:64], 0.0)
            if not do_dve:
                nc.vector.memset(nrm[:, 0:64], 0.0)

        # ---- diagonal (+1) fused into the PSUM drain on the DVE (keeps the
        # ACT queue free for the next iteration's copies), then DMA out
        o_sb = osb.tile([128, 512], f32, tag="osb")
        nc.vector.tensor_scalar_add(o_sb, o_ps[:, :], 1.0)
        nc.sync.dma_start(out=o_d, in_=o_sb)


# --------------------------------------------------------------------------
# host side
# --------------------------------------------------------------------------

def prep_inputs(x, T):
    """Shared (core-independent) device inputs, packed partition-first."""
    xf = np.ascontiguousarray(x.reshape(N, HW, F))
    # tw[f, (fh, c, b)] = T[fh*128+f, b, c]
    tw = T.reshape(FH, 128, B, C).transpose(1, 0, 3, 2)     # f, fh, c, b
    tw_in = np.ascontiguousarray(tw.reshape(128, FH * C * B)).astype(np.float16)
    iw_in = np.eye(128, dtype=np.float16)
    return xf, tw_in, iw_in


def core_in_map(xf, tw_in, iw_in, k):
    xs = xf[:, k * HWL:(k + 1) * HWL, :]          # [i, hwl, f]
    # xT[f, (fh, hh, hlh, i, hl8)] = x[i, hh*16+hlh*8+hl8, fh*128+f]
    xT = xs.reshape(N, HH, 2, 8, FH, 128).transpose(5, 4, 1, 2, 0, 3)
    xT = np.ascontiguousarray(xT.reshape(128, FH * HH * N * HL))
    return {"xT": xT.astype(np.float16), "tw": tw_in, "iw": iw_in}


def gather_ob(core_outs):
    """core_outs: list of 8 arrays [128,512] f32 -> o_b [N,16,16,B]."""
    obs = []
    for res in core_outs:
        v = res.astype(np.float32).reshape(HH, B, 2, N, 8)  # hh,b,hlh,i,hl8
        obs.append(v.transpose(3, 0, 2, 4, 1).reshape(N, HWL, B))
    return np.concatenate(obs, axis=1).reshape(N, 16, 16, B)


def _get_program(reps=1, loop=None, parts="all"):
    key = ("nc", reps, loop, parts)
    if key in _CACHED:
        return _CACHED[key]
    from contextlib import ExitStack
    import concourse.bacc as bacc
    import concourse.mybir as mybir
    import concourse.tile as tile

    _get_ps8_op()
    nc = bacc.Bacc("TRN2", target_bir_lowering=False, debug=False,
                   num_devices=CORES)
    f16, f32 = mybir.dt.float16, mybir.dt.float32
    ins = {
        "xT": nc.dram_tensor("xT", [128, FH * HH * N * HL], f16,
                             kind="ExternalInput").ap(),
        "tw": nc.dram_tensor("tw", [128, FH * C * B], f16,
                             kind="ExternalInput").ap(),
        "iw": nc.dram_tensor("iw", [128, 128], f16,
                             kind="ExternalInput").ap(),
    }
    outs = {
        "o": nc.dram_tensor("o", [128, 512], f32, kind="ExternalOutput").ap(),
    }
    with tile.TileContext(nc) as tc:
        if loop:
            with ExitStack() as ctx:
                pools = make_pools(tc, ctx)
                with tc.For_i(0, loop, 1,
                              hint_engines=(mybir.EngineType.PE,
                                            mybir.EngineType.DVE)):
                    build_body(tc, outs, ins, pools=pools, parts=parts)
        else:
            for r in range(reps):
                build_body(tc, outs, ins, rep=r)
    nc.compile()
    _CACHED[key] = nc
    return nc


def kernel(x, T):
    x = np.asarray(x, dtype=np.float32)
    T = np.asarray(T, dtype=np.float32)
    from concourse.bass_utils import run_bass_kernel_spmd

    nc = _get_program()
    xf, tw_in, iw_in = prep_inputs(x, T)
    in_maps = [core_in_map(xf, tw_in, iw_in, k) for k in range(CORES)]
    res = run_bass_kernel_spmd(nc, in_maps, core_ids=list(range(CORES)))
    ob = gather_ob([r["o"] for r in res.results])
    return np.concatenate([x, ob], axis=3)


# revision 17
# speedup vs baseline: 2.0262x; 1.3732x over previous
"""MiniBatchDiscrimination Trainium2 kernel — DVE-fused-norm version (v4).

reference:
    M = einsum('nhwf,fbc->nhwbc', x, T)          # [N,H,W,B,C]
    norm = sum_c |M[i] - M[j]|                   # [N,N,H,W,B]
    o_b  = sum_j exp(-norm)                      # [N,H,W,B]
    out  = concat([x, o_b], axis=3)              # [N,H,W,F+B]

Sharding: embarrassingly parallel over HW=256 spatial positions; each core
takes a 32-position hw slice and computes all pairs for it.

Core trick: the c-axis lives INNERMOST IN THE FREE DIM so a custom 8-state
DVE micro-op ("PAIRSUM8", 2X_1PORT) computes, per (i,hwl,b) position, the
full L1 norm sum_c |M2[i+d] - M2[i]| in one streaming pass — consuming
2 f16/lane/cycle and emitting one compacted f16 norm per 8 inputs.  This
removes the PE c-reduce matmuls entirely; PE only does the small x@T
projection (stage B) and identity-weight j-sum folds.

  M2   [128 part=(hh2,b64), 6144 free=(i48,hl16,c8)] f16; i in [32,48) is a
       circular pad (copy of i in [0,16)) so every d-stream is contiguous.
  nrm  [128, (d15,i32,hl16 | d16:i16,hl16)] f16 — d=1..15 in giant DVE
       instructions via a 3D AP (outer dim = d, stride one i; in1 broadcast).
  E    exp(-nrm) f16 (ACT)
  o_ps [128 part=(hh,b), 512=(i,hl)] f32 PSUM — j-sum via identity-weight
       matmuls: direct (col) + shifted (col+16d mod 512) accumulation;
       diagonal +1 fused into the drain.

Scheduling: tc.For_i puts an all-engine barrier between iterations, so the
loop body is U manually-unrolled executions, software-pipelined in trace
order: body k's DVE streams while PE/ACT build body k+1's M2 and compute
body k-1's exp/j-sum/drain.  Steady state is DVE-bound.
"""

import os
import sys

for _p in ("/opt/trn_rl_repo", "/opt/pypackages"):
    if _p not in sys.path and os.path.isdir(_p):
        sys.path.append(_p)

import numpy as np

N, HWL, F, B, C = 32, 32, 256, 64, 8
HW = 256
CORES = 8
FH = 2           # f in two partition halves of 128
HH = 2           # hwl_hi: hw position high bit (partition dim)
HL = 16          # hwl_lo: 16 positions (free dim)
IPAD = 48        # i padded 32 -> 48 for circular d-shift reads
UNROLL = 8       # bodies per For_i iteration (amortizes the loop barrier)

_CACHED = {}


# --------------------------------------------------------------------------
# PAIRSUM8 custom DVE op: out[g] = sum_{k<8} |in0[8g+k] - in1[8g+k]|
# 2X_1PORT 8-state machine; emits two group sums (WR0_LO/WR0_HI) every
# 4th pair-cycle; out FD = in FD / 8, contiguous.
# --------------------------------------------------------------------------

def _mk_ps8_2x():
    from concourse.dve_uop import (
        ENABLE, AluInp, AluOp, DelayInp, InpSel, OutPath, OutSel, Trigger,
        UopConfig, UopDpConfig,
    )

    def base(next_idx):
        u = UopConfig()
        u.enable_input(InpSel.SRC_0, 0).enable_input(InpSel.SRC_1, 1)
        u.enable_input(InpSel.SRC_0_HI, 2).enable_input(InpSel.SRC_1_HI, 3)
        u.enable_input(InpSel.ZERO, 4)
        u.require_inp0 = ENABLE
        u.require_inp1 = ENABLE
        u.trigger = (Trigger.SRC_TENSOR_DONE, Trigger.COUNT, Trigger.NONE)
        u.next_uop = (0, next_idx, 0)
        u.repeat_count = 1
        dp = u.datapath_config
        # s0: |a_lo-b_lo|; carry a_hi(d1), b_hi(d2), zero(d3)
        dp[0] = (UopDpConfig()
                 .enable_alu(AluOp.ABSOLUTE_DIFF, AluInp.PREV_ALU_OUT,
                             AluInp.PREV_DELAY_0)
                 .pass_through_delay(1, 2, 3))
        # s1: |a_hi-b_hi|; d0 <- lo result
        dp[1] = (UopDpConfig()
                 .enable_alu(AluOp.ABSOLUTE_DIFF, AluInp.PREV_DELAY_1,
                             AluInp.PREV_DELAY_2)
                 .enable_delay_from_src(DelayInp.PREV_ALU_OUT, 0)
                 .pass_through_delay(3))
        # s2: pair sum s = lo + hi
        dp[2] = (UopDpConfig()
                 .enable_alu(AluOp.ADD, AluInp.PREV_ALU_OUT,
                             AluInp.PREV_DELAY_0)
                 .pass_through_delay(3))
        for i in range(3, 8):
            dp[i] = UopDpConfig().pass_through_alu().pass_through_delay(3)
        return u

    def s0(nx):            # park flopA = s + 0
        u = base(nx)
        u.datapath_config[3] = (UopDpConfig()
                                .enable_alu(AluOp.ADD, AluInp.PREV_ALU_OUT,
                                            AluInp.PREV_DELAY_3)
                                .pass_through_delay(3))
        return u

    def sa(nx):            # flopA += s
        u = base(nx)
        u.datapath_config[3] = (UopDpConfig()
                                .enable_alu(AluOp.ADD, AluInp.PREV_ALU_OUT,
                                            AluInp.CURR_ALU_OUT)
                                .pass_through_delay(3))
        return u

    def s4(nx):            # s3 off (s rides d1); flopB = s + 0
        u = base(nx)
        u.datapath_config[3] = (UopDpConfig()
                                .enable_delay_from_src(DelayInp.PREV_ALU_OUT, 1)
                                .pass_through_delay(3))
        u.datapath_config[4] = (UopDpConfig()
                                .enable_alu(AluOp.ADD, AluInp.PREV_DELAY_1,
                                            AluInp.PREV_DELAY_3)
                                .pass_through_delay(3))
        return u

    def sb(nx):            # flopB += s
        u = base(nx)
        u.datapath_config[3] = (UopDpConfig()
                                .enable_delay_from_src(DelayInp.PREV_ALU_OUT, 1)
                                .pass_through_delay(3))
        u.datapath_config[4] = (UopDpConfig()
                                .enable_alu(AluOp.ADD, AluInp.PREV_DELAY_1,
                                            AluInp.CURR_ALU_OUT)
                                .pass_through_delay(3))
        return u

    def s7(nx):            # emit q0 (flopA export via d2) + q1 (alu)
        u = base(nx)
        u.datapath_config[3] = (UopDpConfig()
                                .enable_alu(AluOp.ADD, AluInp.PREV_DELAY_3,
                                            AluInp.CURR_ALU_OUT)
                                .enable_delay_from_src(DelayInp.PREV_ALU_OUT, 1)
                                .pass_through_delay(3))
        u.datapath_config[4] = (UopDpConfig()
                                .enable_alu(AluOp.ADD, AluInp.PREV_DELAY_1,
                                            AluInp.CURR_ALU_OUT)
                                .enable_delay_from_src(DelayInp.PREV_ALU_OUT, 2)
                                .pass_through_delay(3))
        for i in range(5, 8):
            u.datapath_config[i] = (UopDpConfig().pass_through_alu()
                                    .pass_through_delay(2, 3))
        u.enable_output(OutSel.DELAY_2, OutPath.WR0_LO)
        u.enable_output(OutSel.ALU_OUT, OutPath.WR0_HI)
        return u

    # idx: 0=S0entry 1..7=S1..S7 8=S0loop (0 = IDLE, loop restarts at 8)
    return [s0(1), sa(2), sa(3), sa(4), s4(5), sb(6), sb(7), s7(8), s0(1)]


def _get_ps8_op():
    if "ps8" in _CACHED:
        return _CACHED["ps8"]
    from concourse import dve_ops
    from concourse.dve_spec import Spec, Src0, Src1, maxx
    from concourse.dve_uop import DveOpSpec

    NAME = "PAIRSUM8_ANT"
    for op in dve_ops.OPS:
        if op.name == NAME:
            _CACHED["ps8"] = op
            return op
    spec = Spec(
        body=maxx(Src0 - Src1, Src1 - Src0),
        reference=lambda in0, in1, s0, s1, imm2: np.abs(
            in0.astype(np.float32) - in1.astype(np.float32)),
    )
    op = dve_ops.DveOp(NAME, spec, subdim=False, uops_sha={})
    dve_ops.OPS.append(op)
    dve_ops.CUSTOM_DVE_SPECS[op.name] = op.spec
    row = dve_ops._CUSTOM_DVE_ROW_BASE + len(dve_ops.OPS) - 1
    dve_ops._SUB_OPCODE_FOR_NAME[op.name] = row
    uops = _mk_ps8_2x()
    compiled = DveOpSpec(
        name=NAME, opcode=row, uops=uops, uops_2x=uops,
        perf_max=1, rd1_en=True,
    )
    compiled.validate("v3")
    dve_ops._COMPILE_CACHE[(NAME, "v3")] = compiled
    dve_ops._COMPILE_CACHE[(NAME, "v4")] = compiled
    _CACHED["ps8"] = op
    return op


# --------------------------------------------------------------------------
# device program
# --------------------------------------------------------------------------

def make_pools(tc, ctx, rep=0):
    sfx = f"_{rep}"
    singles = ctx.enter_context(tc.tile_pool(name="singles" + sfx, bufs=1))
    xTp = ctx.enter_context(tc.tile_pool(name="xTp" + sfx, bufs=2))
    m2p = ctx.enter_context(tc.tile_pool(name="m2p" + sfx, bufs=2))
    psB = ctx.enter_context(tc.tile_pool(name="psB" + sfx, bufs=3,
                                         space="PSUM"))
    nrmp = ctx.enter_context(tc.tile_pool(name="nrmp" + sfx, bufs=2))
    Ep = ctx.enter_context(tc.tile_pool(name="Ep" + sfx, bufs=2))
    psO = ctx.enter_context(tc.tile_pool(name="psO" + sfx, bufs=2,
                                         space="PSUM"))
    osb = ctx.enter_context(tc.tile_pool(name="osb" + sfx, bufs=2))
    return singles, xTp, m2p, psB, nrmp, Ep, psO, osb


# d-stream chunks: (d_first, d_last); 16 handled as the half-range stream
CHUNKS = ((1, 8), (9, 15), (16, 16))


def build_block(tc, outs, ins, rep=0, pools=None, unroll=1):
    """Trace `unroll` software-pipelined executions of the kernel body.

    ins:  xT  [128,2048] f16  xT[f,(fh,hh,i,hl)] = x[i, hw(core,hh,hl), fh*128+f]
          tw  [128,1024] f16  tw[f,(fh,c,b)] = T[fh*128+f, b, c]
          iw  [128,128]  f16  identity
    outs: o   [128,512]  f32  o[hh*64+b, i*16+hl] = o_b[i, hw(core,hh,hl), b]
    """
    from contextlib import ExitStack

    import concourse.mybir as mybir

    nc = tc.nc
    f16 = mybir.dt.float16
    f32 = mybir.dt.float32
    ps8 = _get_ps8_op()

    xT_d, tw_d, iw_d = ins["xT"], ins["tw"], ins["iw"]
    o_d = outs["o"]

    with ExitStack() as ctx:
        if pools is None:
            pools = make_pools(tc, ctx, rep)
        singles, xTp, m2p, psB, nrmp, Ep, psO, osb = pools

        tw_t = singles.tile([128, FH * C * B], f16, tag="tw")
        nc.sync.dma_start(out=tw_t, in_=tw_d)
        iw_t = singles.tile([128, 128], f16, tag="iw")
        nc.sync.dma_start(out=iw_t, in_=iw_d)
        tw_s = [[tw_t[:, (fh * C + c) * B:(fh * C + c + 1) * B]
                 for c in range(C)] for fh in range(FH)]

        def load_x(k):
            xT_t = xTp.tile([128, FH * HH * N * HL], f16, tag="xT")
            nc.sync.dma_start(out=xT_t[:, 0:1024], in_=xT_d[:, 0:1024])
            nc.sync.dma_start(out=xT_t[:, 1024:2048], in_=xT_d[:, 1024:2048])
            return [[xT_t[:, (fh * HH + hh) * 512:(fh * HH + hh + 1) * 512]
                     for hh in range(HH)] for fh in range(FH)]

        def stage_b(xT_s):
            """x @ T into the c-interleaved M2 layout; returns the m2 tile."""
            m2 = m2p.tile([128, IPAD * HL * C], f16, tag="m2")
            m2v = m2.rearrange("p (x c) -> p x c", c=C)
            for g in range(4):
                ps = psB.tile([128, 1024], f32, tag="psB")
                for cl in range(2):
                    c = 2 * g + cl
                    for hh in range(HH):
                        for fh in range(FH):
                            nc.tensor.matmul(
                                ps[64 * hh:64 * hh + 64,
                                   cl * 512:(cl + 1) * 512],
                                lhsT=tw_s[fh][c], rhs=xT_s[fh][hh],
                                start=(fh == 0), stop=(fh == 1),
                                tile_position=(0, 64 * hh),
                                skip_group_check=True,
                            )
                # m2[x*8 + c] <- ps[(c2, x512)]
                nc.scalar.copy(
                    out=m2v[:, 0:512, 2 * g:2 * g + 2],
                    in_=ps.rearrange("p (c x) -> p x c", c=2))
            # circular pad: i in [32,48) := i in [0,16)  (ACT)
            nc.scalar.copy(out=m2[:, 4096:6144], in_=m2[:, 0:2048])
            return m2

        def dve_chunks(m2):
            nrm = nrmp.tile([128, 15 * 512 + 256], f16, tag="nrm")
            for d0, d1 in CHUNKS:
                nd = min(d1, 15) - d0 + 1
                if d0 <= 15:
                    in0 = m2[:, 128 * d0:128 * d0 + 4096].unsqueeze(
                        1).to_broadcast([128, nd, 4096]).copy()
                    in0.ap[1] = (128, nd)    # [step, num]: one i per d
                    in1 = m2[:, 0:4096].unsqueeze(1).to_broadcast(
                        [128, nd, 4096])
                    o = nrm[:, (d0 - 1) * 512:(d0 - 1 + nd) * 512]
                    bi = nc.vector._custom_dve(
                        ps8, out=o.rearrange("p (d x) -> p d x", d=nd),
                        in0=in0, in1=in1)
                    bi.ins.perf_max = 1
                else:
                    bi = nc.vector._custom_dve(
                        ps8, out=nrm[:, 7680:7936],
                        in0=m2[:, 2048:4096], in1=m2[:, 0:2048])
                    bi.ins.perf_max = 1
            return nrm

        def exp_chunk(nrm, E, d0, d1):
            a = (d0 - 1) * 512
            b = min(d1, 15) * 512 if d1 < 16 else 7936
            nc.scalar.activation(
                out=E[:, a:b], in_=nrm[:, a:b],
                func=mybir.ActivationFunctionType.Exp, scale=-1.0,
            )

        def jsum_chunk(E, o_ps, d0, d1):
            for d in range(d0, d1 + 1):
                if d < 16:
                    Ed = E[:, (d - 1) * 512:d * 512]
                    sh = 16 * d
                    nc.tensor.matmul(o_ps[:, :], lhsT=iw_t, rhs=Ed,
                                     start=(d == 1), stop=False,
                                     skip_group_check=True)
                    nc.tensor.matmul(o_ps[:, sh:512], lhsT=iw_t,
                                     rhs=Ed[:, 0:512 - sh], start=False,
                                     stop=False, skip_group_check=True)
                    nc.tensor.matmul(o_ps[:, 0:sh], lhsT=iw_t,
                                     rhs=Ed[:, 512 - sh:512], start=False,
                                     stop=False, skip_group_check=True)
                else:
                    E16 = E[:, 7680:7936]
                    nc.tensor.matmul(o_ps[:, 0:256], lhsT=iw_t, rhs=E16,
                                     start=False, stop=False,
                                     skip_group_check=True)
                    nc.tensor.matmul(o_ps[:, 256:512], lhsT=iw_t, rhs=E16,
                                     start=False, stop=True,
                                     skip_group_check=True)

        def finish(nrm):
            """exp + j-sum + drain + out-DMA for one body's norms."""
            E = Ep.tile([128, 15 * 512 + 256], f16, tag="E")
            o_ps = psO.tile([128, 512], f32, tag="oPs")
            for d0, d1 in CHUNKS:
                exp_chunk(nrm, E, d0, d1)
                jsum_chunk(E, o_ps, d0, d1)
            o_sb = osb.tile([128, 512], f32, tag="osb")
            nc.scalar.activation(
                out=o_sb, in_=o_ps[:, :],
                func=mybir.ActivationFunctionType.Identity,
                bias=1.0, scale=1.0,
            )
            nc.sync.dma_start(out=o_d, in_=o_sb)

        # -- software-pipelined unrolled block --
        m2 = stage_b(load_x(0))
        nrm_prev = None
        for k in range(unroll):
            nrm = dve_chunks(m2)
            if k + 1 < unroll:
                m2 = stage_b(load_x(k + 1))
            if nrm_prev is not None:
                finish(nrm_prev)
            nrm_prev = nrm
        finish(nrm_prev)


# --------------------------------------------------------------------------
# host side
# --------------------------------------------------------------------------

def prep_inputs(x, T):
    """Shared (core-independent) device inputs, packed partition-first."""
    xf = np.ascontiguousarray(x.reshape(N, HW, F))
    # tw[f, (fh, c, b)] = T[fh*128+f, b, c]
    tw = T.reshape(FH, 128, B, C).transpose(1, 0, 3, 2)     # f, fh, c, b
    tw_in = np.ascontiguousarray(tw.reshape(128, FH * C * B)).astype(np.float16)
    iw_in = np.eye(128, dtype=np.float16)
    return xf, tw_in, iw_in


def core_in_map(xf, tw_in, iw_in, k):
    xs = xf[:, k * HWL:(k + 1) * HWL, :]          # [i, hwl, f]
    # xT[f, (fh, hh, i, hl)] = x[i, hh*16+hl, fh*128+f]
    xT = xs.reshape(N, HH, HL, FH, 128).transpose(4, 3, 1, 0, 2)
    xT = np.ascontiguousarray(xT.reshape(128, FH * HH * N * HL))
    return {"xT": xT.astype(np.float16), "tw": tw_in, "iw": iw_in}


def gather_ob(core_outs):
    """core_outs: list of 8 arrays [128,512] f32 -> o_b [N,16,16,B]."""
    obs = []
    for res in core_outs:
        v = res.astype(np.float32).reshape(HH, B, N, HL)   # hh, b, i, hl
        obs.append(v.transpose(2, 0, 3, 1).reshape(N, HWL, B))  # i, hwl, b
    return np.concatenate(obs, axis=1).reshape(N, 16, 16, B)


def _get_program(reps=1, loop=None, unroll=UNROLL):
    key = ("nc", reps, loop, unroll)
    if key in _CACHED:
        return _CACHED[key]
    from contextlib import ExitStack
    import concourse.bacc as bacc
    import concourse.mybir as mybir
    import concourse.tile as tile

    _get_ps8_op()
    nc = bacc.Bacc("TRN2", target_bir_lowering=False, debug=False,
                   num_devices=CORES)
    f16, f32 = mybir.dt.float16, mybir.dt.float32
    ins = {
        "xT": nc.dram_tensor("xT", [128, FH * HH * N * HL], f16,
                             kind="ExternalInput").ap(),
        "tw": nc.dram_tensor("tw", [128, FH * C * B], f16,
                             kind="ExternalInput").ap(),
        "iw": nc.dram_tensor("iw", [128, 128], f16,
                             kind="ExternalInput").ap(),
    }
    outs = {
        "o": nc.dram_tensor("o", [128, 512], f32, kind="ExternalOutput").ap(),
    }
    with tile.TileContext(nc) as tc:
        if loop:
            with ExitStack() as ctx:
                pools = make_pools(tc, ctx)
                with tc.For_i(0, loop, 1,
                              hint_engines=(mybir.EngineType.PE,
                                            mybir.EngineType.DVE)):
                    build_block(tc, outs, ins, pools=pools, unroll=unroll)
        else:
            for r in range(reps):
                build_block(tc, outs, ins, rep=r, unroll=1)
    nc.compile()
    _CACHED[key] = nc
    return nc


def kernel(x, T):
    x = np.asarray(x, dtype=np.float32)
    T = np.asarray(T, dtype=np.float32)
    from concourse.bass_utils import run_bass_kernel_spmd

    nc = _get_program()
    xf, tw_in, iw_in = prep_inputs(x, T)
    in_maps = [core_in_map(xf, tw_in, iw_in, k) for k in range(CORES)]
    res = run_bass_kernel_spmd(nc, in_maps, core_ids=list(range(CORES)))
    ob = gather_ob([r["o"] for r in res.results])
    return np.concatenate([x, ob], axis=3)


# revision 18
# speedup vs baseline: 2.0506x; 1.0120x over previous
"""MiniBatchDiscrimination Trainium2 kernel — DVE-fused-norm version (v4).

reference:
    M = einsum('nhwf,fbc->nhwbc', x, T)          # [N,H,W,B,C]
    norm = sum_c |M[i] - M[j]|                   # [N,N,H,W,B]
    o_b  = sum_j exp(-norm)                      # [N,H,W,B]
    out  = concat([x, o_b], axis=3)              # [N,H,W,F+B]

Sharding: embarrassingly parallel over HW=256 spatial positions; each core
takes a 32-position hw slice and computes all pairs for it.

Core trick: the c-axis lives INNERMOST IN THE FREE DIM so a custom 8-state
DVE micro-op ("PAIRSUM8", 2X_1PORT) computes, per (i,hwl,b) position, the
full L1 norm sum_c |M2[i+d] - M2[i]| in one streaming pass — consuming
2 f16/lane/cycle and emitting one compacted f16 norm per 8 inputs.  This
removes the PE c-reduce matmuls entirely; PE only does the small x@T
projection (stage B) and identity-weight j-sum folds.

  M2   [128 part=(hh2,b64), 6144 free=(i48,hl16,c8)] f16; i in [32,48) is a
       circular pad (copy of i in [0,16)) so every d-stream is contiguous.
  nrm  [128, (d15,i32,hl16 | d16:i16,hl16)] f16 — d=1..15 in giant DVE
       instructions via a 3D AP (outer dim = d, stride one i; in1 broadcast).
  E    exp(-nrm) f16 (ACT)
  o_ps [128 part=(hh,b), 512=(i,hl)] f32 PSUM — j-sum via identity-weight
       matmuls: direct (col) + shifted (col+16d mod 512) accumulation;
       diagonal +1 fused into the drain.

Scheduling: tc.For_i puts an all-engine barrier between iterations, so the
loop body is U manually-unrolled executions, software-pipelined in trace
order: body k's DVE streams while PE/ACT build body k+1's M2 and compute
body k-1's exp/j-sum/drain.  Steady state is DVE-bound.
"""

import os
import sys

for _p in ("/opt/trn_rl_repo", "/opt/pypackages"):
    if _p not in sys.path and os.path.isdir(_p):
        sys.path.append(_p)

import numpy as np

N, HWL, F, B, C = 32, 32, 256, 64, 8
HW = 256
CORES = 8
FH = 2           # f in two partition halves of 128
HH = 2           # hwl_hi: hw position high bit (partition dim)
HL = 16          # hwl_lo: 16 positions (free dim)
IPAD = 48        # i padded 32 -> 48 for circular d-shift reads
UNROLL = 12      # bodies per For_i iteration (amortizes the loop barrier)

_CACHED = {}


# --------------------------------------------------------------------------
# PAIRSUM8 custom DVE op: out[g] = sum_{k<8} |in0[8g+k] - in1[8g+k]|
# 2X_1PORT 8-state machine; emits two group sums (WR0_LO/WR0_HI) every
# 4th pair-cycle; out FD = in FD / 8, contiguous.
# --------------------------------------------------------------------------

def _mk_ps8_2x():
    from concourse.dve_uop import (
        ENABLE, AluInp, AluOp, DelayInp, InpSel, OutPath, OutSel, Trigger,
        UopConfig, UopDpConfig,
    )

    def base(next_idx):
        u = UopConfig()
        u.enable_input(InpSel.SRC_0, 0).enable_input(InpSel.SRC_1, 1)
        u.enable_input(InpSel.SRC_0_HI, 2).enable_input(InpSel.SRC_1_HI, 3)
        u.enable_input(InpSel.ZERO, 4)
        u.require_inp0 = ENABLE
        u.require_inp1 = ENABLE
        u.trigger = (Trigger.SRC_TENSOR_DONE, Trigger.COUNT, Trigger.NONE)
        u.next_uop = (0, next_idx, 0)
        u.repeat_count = 1
        dp = u.datapath_config
        # s0: |a_lo-b_lo|; carry a_hi(d1), b_hi(d2), zero(d3)
        dp[0] = (UopDpConfig()
                 .enable_alu(AluOp.ABSOLUTE_DIFF, AluInp.PREV_ALU_OUT,
                             AluInp.PREV_DELAY_0)
                 .pass_through_delay(1, 2, 3))
        # s1: |a_hi-b_hi|; d0 <- lo result
        dp[1] = (UopDpConfig()
                 .enable_alu(AluOp.ABSOLUTE_DIFF, AluInp.PREV_DELAY_1,
                             AluInp.PREV_DELAY_2)
                 .enable_delay_from_src(DelayInp.PREV_ALU_OUT, 0)
                 .pass_through_delay(3))
        # s2: pair sum s = lo + hi
        dp[2] = (UopDpConfig()
                 .enable_alu(AluOp.ADD, AluInp.PREV_ALU_OUT,
                             AluInp.PREV_DELAY_0)
                 .pass_through_delay(3))
        for i in range(3, 8):
            dp[i] = UopDpConfig().pass_through_alu().pass_through_delay(3)
        return u

    def s0(nx):            # park flopA = s + 0
        u = base(nx)
        u.datapath_config[3] = (UopDpConfig()
                                .enable_alu(AluOp.ADD, AluInp.PREV_ALU_OUT,
                                            AluInp.PREV_DELAY_3)
                                .pass_through_delay(3))
        return u

    def sa(nx):            # flopA += s
        u = base(nx)
        u.datapath_config[3] = (UopDpConfig()
                                .enable_alu(AluOp.ADD, AluInp.PREV_ALU_OUT,
                                            AluInp.CURR_ALU_OUT)
                                .pass_through_delay(3))
        return u

    def s4(nx):            # s3 off (s rides d1); flopB = s + 0
        u = base(nx)
        u.datapath_config[3] = (UopDpConfig()
                                .enable_delay_from_src(DelayInp.PREV_ALU_OUT, 1)
                                .pass_through_delay(3))
        u.datapath_config[4] = (UopDpConfig()
                                .enable_alu(AluOp.ADD, AluInp.PREV_DELAY_1,
                                            AluInp.PREV_DELAY_3)
                                .pass_through_delay(3))
        return u

    def sb(nx):            # flopB += s
        u = base(nx)
        u.datapath_config[3] = (UopDpConfig()
                                .enable_delay_from_src(DelayInp.PREV_ALU_OUT, 1)
                                .pass_through_delay(3))
        u.datapath_config[4] = (UopDpConfig()
                                .enable_alu(AluOp.ADD, AluInp.PREV_DELAY_1,
                                            AluInp.CURR_ALU_OUT)
                                .pass_through_delay(3))
        return u

    def s7(nx):            # emit q0 (flopA export via d2) + q1 (alu)
        u = base(nx)
        u.datapath_config[3] = (UopDpConfig()
                                .enable_alu(AluOp.ADD, AluInp.PREV_DELAY_3,
                                            AluInp.CURR_ALU_OUT)
                                .enable_delay_from_src(DelayInp.PREV_ALU_OUT, 1)
                                .pass_through_delay(3))
        u.datapath_config[4] = (UopDpConfig()
                                .enable_alu(AluOp.ADD, AluInp.PREV_DELAY_1,
                                            AluInp.CURR_ALU_OUT)
                                .enable_delay_from_src(DelayInp.PREV_ALU_OUT, 2)
                                .pass_through_delay(3))
        for i in range(5, 8):
            u.datapath_config[i] = (UopDpConfig().pass_through_alu()
                                    .pass_through_delay(2, 3))
        u.enable_output(OutSel.DELAY_2, OutPath.WR0_LO)
        u.enable_output(OutSel.ALU_OUT, OutPath.WR0_HI)
        return u

    # idx: 0=S0entry 1..7=S1..S7 8=S0loop (0 = IDLE, loop restarts at 8)
    return [s0(1), sa(2), sa(3), sa(4), s4(5), sb(6), sb(7), s7(8), s0(1)]


def _get_ps8_op():
    if "ps8" in _CACHED:
        return _CACHED["ps8"]
    from concourse import dve_ops
    from concourse.dve_spec import Spec, Src0, Src1, maxx
    from concourse.dve_uop import DveOpSpec

    NAME = "PAIRSUM8_ANT"
    for op in dve_ops.OPS:
        if op.name == NAME:
            _CACHED["ps8"] = op
            return op
    spec = Spec(
        body=maxx(Src0 - Src1, Src1 - Src0),
        reference=lambda in0, in1, s0, s1, imm2: np.abs(
            in0.astype(np.float32) - in1.astype(np.float32)),
    )
    op = dve_ops.DveOp(NAME, spec, subdim=False, uops_sha={})
    dve_ops.OPS.append(op)
    dve_ops.CUSTOM_DVE_SPECS[op.name] = op.spec
    row = dve_ops._CUSTOM_DVE_ROW_BASE + len(dve_ops.OPS) - 1
    dve_ops._SUB_OPCODE_FOR_NAME[op.name] = row
    uops = _mk_ps8_2x()
    compiled = DveOpSpec(
        name=NAME, opcode=row, uops=uops, uops_2x=uops,
        perf_max=1, rd1_en=True,
    )
    compiled.validate("v3")
    dve_ops._COMPILE_CACHE[(NAME, "v3")] = compiled
    dve_ops._COMPILE_CACHE[(NAME, "v4")] = compiled
    _CACHED["ps8"] = op
    return op


# --------------------------------------------------------------------------
# device program
# --------------------------------------------------------------------------

def make_pools(tc, ctx, rep=0):
    sfx = f"_{rep}"
    singles = ctx.enter_context(tc.tile_pool(name="singles" + sfx, bufs=1))
    xTp = ctx.enter_context(tc.tile_pool(name="xTp" + sfx, bufs=2))
    m2p = ctx.enter_context(tc.tile_pool(name="m2p" + sfx, bufs=2))
    psB = ctx.enter_context(tc.tile_pool(name="psB" + sfx, bufs=3,
                                         space="PSUM"))
    nrmp = ctx.enter_context(tc.tile_pool(name="nrmp" + sfx, bufs=2))
    Ep = ctx.enter_context(tc.tile_pool(name="Ep" + sfx, bufs=2))
    psO = ctx.enter_context(tc.tile_pool(name="psO" + sfx, bufs=2,
                                         space="PSUM"))
    osb = ctx.enter_context(tc.tile_pool(name="osb" + sfx, bufs=2))
    return singles, xTp, m2p, psB, nrmp, Ep, psO, osb


# d-stream chunks: (d_first, d_last); 16 handled as the half-range stream
CHUNKS = ((1, 8), (9, 15), (16, 16))       # exp/j-sum granularity
DVE_CHUNKS = ((1, 15), (16, 16))           # DVE instruction granularity


def build_block(tc, outs, ins, rep=0, pools=None, unroll=1):
    """Trace `unroll` software-pipelined executions of the kernel body.

    ins:  xT  [128,2048] f16  xT[f,(fh,hh,i,hl)] = x[i, hw(core,hh,hl), fh*128+f]
          tw  [128,1024] f16  tw[f,(fh,c,b)] = T[fh*128+f, b, c]
          iw  [128,128]  f16  identity
    outs: o   [128,512]  f32  o[hh*64+b, i*16+hl] = o_b[i, hw(core,hh,hl), b]
    """
    from contextlib import ExitStack

    import concourse.mybir as mybir

    nc = tc.nc
    f16 = mybir.dt.float16
    f32 = mybir.dt.float32
    ps8 = _get_ps8_op()

    xT_d, tw_d, iw_d = ins["xT"], ins["tw"], ins["iw"]
    o_d = outs["o"]

    with ExitStack() as ctx:
        if pools is None:
            pools = make_pools(tc, ctx, rep)
        singles, xTp, m2p, psB, nrmp, Ep, psO, osb = pools

        tw_t = singles.tile([128, FH * C * B], f16, tag="tw")
        nc.sync.dma_start(out=tw_t, in_=tw_d)
        iw_t = singles.tile([128, 128], f16, tag="iw")
        nc.sync.dma_start(out=iw_t, in_=iw_d)
        tw_s = [[tw_t[:, (fh * C + c) * B:(fh * C + c + 1) * B]
                 for c in range(C)] for fh in range(FH)]

        def load_x(k):
            xT_t = xTp.tile([128, FH * HH * N * HL], f16, tag="xT")
            nc.sync.dma_start(out=xT_t[:, 0:1024], in_=xT_d[:, 0:1024])
            nc.sync.dma_start(out=xT_t[:, 1024:2048], in_=xT_d[:, 1024:2048])
            return [[xT_t[:, (fh * HH + hh) * 512:(fh * HH + hh + 1) * 512]
                     for hh in range(HH)] for fh in range(FH)]

        def stage_b(xT_s):
            """x @ T into the c-interleaved M2 layout; returns the m2 tile."""
            m2 = m2p.tile([128, IPAD * HL * C], f16, tag="m2")
            m2v = m2.rearrange("p (x c) -> p x c", c=C)
            for g in range(4):
                ps = psB.tile([128, 1024], f32, tag="psB")
                for cl in range(2):
                    c = 2 * g + cl
                    for hh in range(HH):
                        for fh in range(FH):
                            nc.tensor.matmul(
                                ps[64 * hh:64 * hh + 64,
                                   cl * 512:(cl + 1) * 512],
                                lhsT=tw_s[fh][c], rhs=xT_s[fh][hh],
                                start=(fh == 0), stop=(fh == 1),
                                tile_position=(0, 64 * hh),
                                skip_group_check=True,
                            )
                # m2[x*8 + c] <- ps[(c2, x512)]
                nc.scalar.copy(
                    out=m2v[:, 0:512, 2 * g:2 * g + 2],
                    in_=ps.rearrange("p (c x) -> p x c", c=2))
            # circular pad: i in [32,48) := i in [0,16)  (ACT)
            nc.scalar.copy(out=m2[:, 4096:6144], in_=m2[:, 0:2048])
            return m2

        def dve_chunks(m2):
            nrm = nrmp.tile([128, 15 * 512 + 256], f16, tag="nrm")
            for d0, d1 in DVE_CHUNKS:
                nd = min(d1, 15) - d0 + 1
                if d0 <= 15:
                    in0 = m2[:, 128 * d0:128 * d0 + 4096].unsqueeze(
                        1).to_broadcast([128, nd, 4096]).copy()
                    in0.ap[1] = (128, nd)    # [step, num]: one i per d
                    in1 = m2[:, 0:4096].unsqueeze(1).to_broadcast(
                        [128, nd, 4096])
                    o = nrm[:, (d0 - 1) * 512:(d0 - 1 + nd) * 512]
                    bi = nc.vector._custom_dve(
                        ps8, out=o.rearrange("p (d x) -> p d x", d=nd),
                        in0=in0, in1=in1)
                    bi.ins.perf_max = 1
                else:
                    bi = nc.vector._custom_dve(
                        ps8, out=nrm[:, 7680:7936],
                        in0=m2[:, 2048:4096], in1=m2[:, 0:2048])
                    bi.ins.perf_max = 1
            return nrm

        def exp_chunk(nrm, E, d0, d1):
            a = (d0 - 1) * 512
            b = min(d1, 15) * 512 if d1 < 16 else 7936
            nc.scalar.activation(
                out=E[:, a:b], in_=nrm[:, a:b],
                func=mybir.ActivationFunctionType.Exp, scale=-1.0,
            )

        def jsum_chunk(E, o_ps, d0, d1):
            for d in range(d0, d1 + 1):
                if d < 16:
                    Ed = E[:, (d - 1) * 512:d * 512]
                    sh = 16 * d
                    nc.tensor.matmul(o_ps[:, :], lhsT=iw_t, rhs=Ed,
                                     start=(d == 1), stop=False,
                                     skip_group_check=True)
                    nc.tensor.matmul(o_ps[:, sh:512], lhsT=iw_t,
                                     rhs=Ed[:, 0:512 - sh], start=False,
                                     stop=False, skip_group_check=True)
                    nc.tensor.matmul(o_ps[:, 0:sh], lhsT=iw_t,
                                     rhs=Ed[:, 512 - sh:512], start=False,
                                     stop=False, skip_group_check=True)
                else:
                    E16 = E[:, 7680:7936]
                    nc.tensor.matmul(o_ps[:, 0:256], lhsT=iw_t, rhs=E16,
                                     start=False, stop=False,
                                     skip_group_check=True)
                    nc.tensor.matmul(o_ps[:, 256:512], lhsT=iw_t, rhs=E16,
                                     start=False, stop=True,
                                     skip_group_check=True)

        def finish(nrm):
            """exp + j-sum + drain + out-DMA for one body's norms."""
            E = Ep.tile([128, 15 * 512 + 256], f16, tag="E")
            o_ps = psO.tile([128, 512], f32, tag="oPs")
            for d0, d1 in CHUNKS:
                exp_chunk(nrm, E, d0, d1)
                jsum_chunk(E, o_ps, d0, d1)
            o_sb = osb.tile([128, 512], f32, tag="osb")
            nc.scalar.activation(
                out=o_sb, in_=o_ps[:, :],
                func=mybir.ActivationFunctionType.Identity,
                bias=1.0, scale=1.0,
            )
            nc.sync.dma_start(out=o_d, in_=o_sb)

        # -- software-pipelined unrolled block --
        m2 = stage_b(load_x(0))
        nrm_prev = None
        for k in range(unroll):
            nrm = dve_chunks(m2)
            if k + 1 < unroll:
                m2 = stage_b(load_x(k + 1))
            if nrm_prev is not None:
                finish(nrm_prev)
            nrm_prev = nrm
        finish(nrm_prev)


# --------------------------------------------------------------------------
# host side
# --------------------------------------------------------------------------

def prep_inputs(x, T):
    """Shared (core-independent) device inputs, packed partition-first."""
    xf = np.ascontiguousarray(x.reshape(N, HW, F))
    # tw[f, (fh, c, b)] = T[fh*128+f, b, c]
    tw = T.reshape(FH, 128, B, C).transpose(1, 0, 3, 2)     # f, fh, c, b
    tw_in = np.ascontiguousarray(tw.reshape(128, FH * C * B)).astype(np.float16)
    iw_in = np.eye(128, dtype=np.float16)
    return xf, tw_in, iw_in


def core_in_map(xf, tw_in, iw_in, k):
    xs = xf[:, k * HWL:(k + 1) * HWL, :]          # [i, hwl, f]
    # xT[f, (fh, hh, i, hl)] = x[i, hh*16+hl, fh*128+f]
    xT = xs.reshape(N, HH, HL, FH, 128).transpose(4, 3, 1, 0, 2)
    xT = np.ascontiguousarray(xT.reshape(128, FH * HH * N * HL))
    return {"xT": xT.astype(np.float16), "tw": tw_in, "iw": iw_in}


def gather_ob(core_outs):
    """core_outs: list of 8 arrays [128,512] f32 -> o_b [N,16,16,B]."""
    obs = []
    for res in core_outs:
        v = res.astype(np.float32).reshape(HH, B, N, HL)   # hh, b, i, hl
        obs.append(v.transpose(2, 0, 3, 1).reshape(N, HWL, B))  # i, hwl, b
    return np.concatenate(obs, axis=1).reshape(N, 16, 16, B)


def _get_program(reps=1, loop=None, unroll=UNROLL):
    key = ("nc", reps, loop, unroll)
    if key in _CACHED:
        return _CACHED[key]
    from contextlib import ExitStack
    import concourse.bacc as bacc
    import concourse.mybir as mybir
    import concourse.tile as tile

    _get_ps8_op()
    nc = bacc.Bacc("TRN2", target_bir_lowering=False, debug=False,
                   num_devices=CORES)
    f16, f32 = mybir.dt.float16, mybir.dt.float32
    ins = {
        "xT": nc.dram_tensor("xT", [128, FH * HH * N * HL], f16,
                             kind="ExternalInput").ap(),
        "tw": nc.dram_tensor("tw", [128, FH * C * B], f16,
                             kind="ExternalInput").ap(),
        "iw": nc.dram_tensor("iw", [128, 128], f16,
                             kind="ExternalInput").ap(),
    }
    outs = {
        "o": nc.dram_tensor("o", [128, 512], f32, kind="ExternalOutput").ap(),
    }
    with tile.TileContext(nc) as tc:
        if loop:
            with ExitStack() as ctx:
                pools = make_pools(tc, ctx)
                with tc.For_i(0, loop, 1,
                              hint_engines=(mybir.EngineType.PE,
                                            mybir.EngineType.DVE)):
                    build_block(tc, outs, ins, pools=pools, unroll=unroll)
        else:
            for r in range(reps):
                build_block(tc, outs, ins, rep=r, unroll=1)
    nc.compile()
    _CACHED[key] = nc
    return nc


def kernel(x, T):
    x = np.asarray(x, dtype=np.float32)
    T = np.asarray(T, dtype=np.float32)
    from concourse.bass_utils import run_bass_kernel_spmd

    nc = _get_program()
    xf, tw_in, iw_in = prep_inputs(x, T)
    in_maps = [core_in_map(xf, tw_in, iw_in, k) for k in range(CORES)]
    res = run_bass_kernel_spmd(nc, in_maps, core_ids=list(range(CORES)))
    ob = gather_ob([r["o"] for r in res.results])
    return np.concatenate([x, ob], axis=3)


# revision 19
# speedup vs baseline: 2.1354x; 1.0413x over previous
"""MiniBatchDiscrimination Trainium2 kernel — DVE-fused-norm version (v4).

reference:
    M = einsum('nhwf,fbc->nhwbc', x, T)          # [N,H,W,B,C]
    norm = sum_c |M[i] - M[j]|                   # [N,N,H,W,B]
    o_b  = sum_j exp(-norm)                      # [N,H,W,B]
    out  = concat([x, o_b], axis=3)              # [N,H,W,F+B]

Sharding: embarrassingly parallel over HW=256 spatial positions; each core
takes a 32-position hw slice and computes all pairs for it.

Core trick: the c-axis lives INNERMOST IN THE FREE DIM so a custom 8-state
DVE micro-op ("PAIRSUM8", 2X_1PORT) computes, per (i,hwl,b) position, the
full L1 norm sum_c |M2[i+d] - M2[i]| in one streaming pass — consuming
2 f16/lane/cycle and emitting one compacted f16 norm per 8 inputs.  This
removes the PE c-reduce matmuls entirely; PE only does the small x@T
projection (stage B) and identity-weight j-sum folds.

  M2   [128 part=(hh2,b64), 6144 free=(i48,hl16,c8)] f16; i in [32,48) is a
       circular pad (copy of i in [0,16)) so every d-stream is contiguous.
  nrm  [128, (d15,i32,hl16 | d16:i16,hl16)] f16 — d=1..15 in giant DVE
       instructions via a 3D AP (outer dim = d, stride one i; in1 broadcast).
  E    exp(-nrm) f16 (ACT)
  o_ps [128 part=(hh,b), 512=(i,hl)] f32 PSUM — j-sum via identity-weight
       matmuls: direct (col) + shifted (col+16d mod 512) accumulation;
       diagonal +1 fused into the drain.

Scheduling: tc.For_i puts an all-engine barrier between iterations, so the
loop body is U manually-unrolled executions, software-pipelined in trace
order: body k's DVE streams while PE/ACT build body k+1's M2 and compute
body k-1's exp/j-sum/drain.  Steady state is DVE-bound.
"""

import os
import sys

for _p in ("/opt/trn_rl_repo", "/opt/pypackages"):
    if _p not in sys.path and os.path.isdir(_p):
        sys.path.append(_p)

import numpy as np

N, HWL, F, B, C = 32, 32, 256, 64, 8
HW = 256
CORES = 8
FH = 2           # f in two partition halves of 128
HH = 2           # hwl_hi: hw position high bit (partition dim)
HL = 16          # hwl_lo: 16 positions (free dim)
IPAD = 48        # i padded 32 -> 48 for circular d-shift reads
UNROLL = 16      # bodies per For_i iteration (amortizes the loop barrier)

_CACHED = {}


# --------------------------------------------------------------------------
# PAIRSUM8 custom DVE op: out[g] = sum_{k<8} |in0[8g+k] - in1[8g+k]|
# 2X_1PORT 8-state machine; emits two group sums (WR0_LO/WR0_HI) every
# 4th pair-cycle; out FD = in FD / 8, contiguous.
# --------------------------------------------------------------------------

def _mk_ps8_2x():
    from concourse.dve_uop import (
        ENABLE, AluInp, AluOp, DelayInp, InpSel, OutPath, OutSel, Trigger,
        UopConfig, UopDpConfig,
    )

    def base(next_idx):
        u = UopConfig()
        u.enable_input(InpSel.SRC_0, 0).enable_input(InpSel.SRC_1, 1)
        u.enable_input(InpSel.SRC_0_HI, 2).enable_input(InpSel.SRC_1_HI, 3)
        u.enable_input(InpSel.ZERO, 4)
        u.require_inp0 = ENABLE
        u.require_inp1 = ENABLE
        u.trigger = (Trigger.SRC_TENSOR_DONE, Trigger.COUNT, Trigger.NONE)
        u.next_uop = (0, next_idx, 0)
        u.repeat_count = 1
        dp = u.datapath_config
        # s0: |a_lo-b_lo|; carry a_hi(d1), b_hi(d2), zero(d3)
        dp[0] = (UopDpConfig()
                 .enable_alu(AluOp.ABSOLUTE_DIFF, AluInp.PREV_ALU_OUT,
                             AluInp.PREV_DELAY_0)
                 .pass_through_delay(1, 2, 3))
        # s1: |a_hi-b_hi|; d0 <- lo result
        dp[1] = (UopDpConfig()
                 .enable_alu(AluOp.ABSOLUTE_DIFF, AluInp.PREV_DELAY_1,
                             AluInp.PREV_DELAY_2)
                 .enable_delay_from_src(DelayInp.PREV_ALU_OUT, 0)
                 .pass_through_delay(3))
        # s2: pair sum s = lo + hi
        dp[2] = (UopDpConfig()
                 .enable_alu(AluOp.ADD, AluInp.PREV_ALU_OUT,
                             AluInp.PREV_DELAY_0)
                 .pass_through_delay(3))
        for i in range(3, 8):
            dp[i] = UopDpConfig().pass_through_alu().pass_through_delay(3)
        return u

    def s0(nx):            # park flopA = s + 0
        u = base(nx)
        u.datapath_config[3] = (UopDpConfig()
                                .enable_alu(AluOp.ADD, AluInp.PREV_ALU_OUT,
                                            AluInp.PREV_DELAY_3)
                                .pass_through_delay(3))
        return u

    def sa(nx):            # flopA += s
        u = base(nx)
        u.datapath_config[3] = (UopDpConfig()
                                .enable_alu(AluOp.ADD, AluInp.PREV_ALU_OUT,
                                            AluInp.CURR_ALU_OUT)
                                .pass_through_delay(3))
        return u

    def s4(nx):            # s3 off (s rides d1); flopB = s + 0
        u = base(nx)
        u.datapath_config[3] = (UopDpConfig()
                                .enable_delay_from_src(DelayInp.PREV_ALU_OUT, 1)
                                .pass_through_delay(3))
        u.datapath_config[4] = (UopDpConfig()
                                .enable_alu(AluOp.ADD, AluInp.PREV_DELAY_1,
                                            AluInp.PREV_DELAY_3)
                                .pass_through_delay(3))
        return u

    def sb(nx):            # flopB += s
        u = base(nx)
        u.datapath_config[3] = (UopDpConfig()
                                .enable_delay_from_src(DelayInp.PREV_ALU_OUT, 1)
                                .pass_through_delay(3))
        u.datapath_config[4] = (UopDpConfig()
                                .enable_alu(AluOp.ADD, AluInp.PREV_DELAY_1,
                                            AluInp.CURR_ALU_OUT)
                                .pass_through_delay(3))
        return u

    def s7(nx):            # emit q0 (flopA export via d2) + q1 (alu)
        u = base(nx)
        u.datapath_config[3] = (UopDpConfig()
                                .enable_alu(AluOp.ADD, AluInp.PREV_DELAY_3,
                                            AluInp.CURR_ALU_OUT)
                                .enable_delay_from_src(DelayInp.PREV_ALU_OUT, 1)
                                .pass_through_delay(3))
        u.datapath_config[4] = (UopDpConfig()
                                .enable_alu(AluOp.ADD, AluInp.PREV_DELAY_1,
                                            AluInp.CURR_ALU_OUT)
                                .enable_delay_from_src(DelayInp.PREV_ALU_OUT, 2)
                                .pass_through_delay(3))
        for i in range(5, 8):
            u.datapath_config[i] = (UopDpConfig().pass_through_alu()
                                    .pass_through_delay(2, 3))
        u.enable_output(OutSel.DELAY_2, OutPath.WR0_LO)
        u.enable_output(OutSel.ALU_OUT, OutPath.WR0_HI)
        return u

    # idx: 0=S0entry 1..7=S1..S7 8=S0loop (0 = IDLE, loop restarts at 8)
    return [s0(1), sa(2), sa(3), sa(4), s4(5), sb(6), sb(7), s7(8), s0(1)]


def _get_ps8_op():
    if "ps8" in _CACHED:
        return _CACHED["ps8"]
    from concourse import dve_ops
    from concourse.dve_spec import Spec, Src0, Src1, maxx
    from concourse.dve_uop import DveOpSpec

    NAME = "PAIRSUM8_ANT"
    for op in dve_ops.OPS:
        if op.name == NAME:
            _CACHED["ps8"] = op
            return op
    spec = Spec(
        body=maxx(Src0 - Src1, Src1 - Src0),
        reference=lambda in0, in1, s0, s1, imm2: np.abs(
            in0.astype(np.float32) - in1.astype(np.float32)),
    )
    op = dve_ops.DveOp(NAME, spec, subdim=False, uops_sha={})
    dve_ops.OPS.append(op)
    dve_ops.CUSTOM_DVE_SPECS[op.name] = op.spec
    row = dve_ops._CUSTOM_DVE_ROW_BASE + len(dve_ops.OPS) - 1
    dve_ops._SUB_OPCODE_FOR_NAME[op.name] = row
    uops = _mk_ps8_2x()
    compiled = DveOpSpec(
        name=NAME, opcode=row, uops=uops, uops_2x=uops,
        perf_max=1, rd1_en=True,
    )
    compiled.validate("v3")
    dve_ops._COMPILE_CACHE[(NAME, "v3")] = compiled
    dve_ops._COMPILE_CACHE[(NAME, "v4")] = compiled
    _CACHED["ps8"] = op
    return op


# --------------------------------------------------------------------------
# device program
# --------------------------------------------------------------------------

def make_pools(tc, ctx, rep=0):
    sfx = f"_{rep}"
    singles = ctx.enter_context(tc.tile_pool(name="singles" + sfx, bufs=1))
    xTp = ctx.enter_context(tc.tile_pool(name="xTp" + sfx, bufs=2))
    m2p = ctx.enter_context(tc.tile_pool(name="m2p" + sfx, bufs=2))
    psB = ctx.enter_context(tc.tile_pool(name="psB" + sfx, bufs=3,
                                         space="PSUM"))
    nrmp = ctx.enter_context(tc.tile_pool(name="nrmp" + sfx, bufs=2))
    Ep = ctx.enter_context(tc.tile_pool(name="Ep" + sfx, bufs=2))
    psO = ctx.enter_context(tc.tile_pool(name="psO" + sfx, bufs=2,
                                         space="PSUM"))
    osb = ctx.enter_context(tc.tile_pool(name="osb" + sfx, bufs=2))
    return singles, xTp, m2p, psB, nrmp, Ep, psO, osb


# d-stream chunks: (d_first, d_last); 16 handled as the half-range stream
CHUNKS = ((1, 8), (9, 15), (16, 16))       # exp/j-sum granularity
DVE_CHUNKS = ((1, 15), (16, 16))           # DVE instruction granularity


def build_block(tc, outs, ins, rep=0, pools=None, unroll=1):
    """Trace `unroll` software-pipelined executions of the kernel body.

    ins:  xT  [128,2048] f16  xT[f,(fh,hh,i,hl)] = x[i, hw(core,hh,hl), fh*128+f]
          tw  [128,1024] f16  tw[f,(fh,c,b)] = T[fh*128+f, b, c]
          iw  [128,128]  f16  identity
    outs: o   [128,512]  f32  o[hh*64+b, i*16+hl] = o_b[i, hw(core,hh,hl), b]
    """
    from contextlib import ExitStack

    import concourse.mybir as mybir

    nc = tc.nc
    f16 = mybir.dt.float16
    f32 = mybir.dt.float32
    ps8 = _get_ps8_op()

    xT_d, tw_d, iw_d = ins["xT"], ins["tw"], ins["iw"]
    o_d = outs["o"]

    with ExitStack() as ctx:
        if pools is None:
            pools = make_pools(tc, ctx, rep)
        singles, xTp, m2p, psB, nrmp, Ep, psO, osb = pools

        tw_t = singles.tile([128, FH * C * B], f16, tag="tw")
        nc.sync.dma_start(out=tw_t, in_=tw_d)
        iw_t = singles.tile([128, 128], f16, tag="iw")
        nc.sync.dma_start(out=iw_t, in_=iw_d)
        tw_s = [[tw_t[:, (fh * C + c) * B:(fh * C + c + 1) * B]
                 for c in range(C)] for fh in range(FH)]

        def load_x(k):
            xT_t = xTp.tile([128, FH * HH * N * HL], f16, tag="xT")
            nc.sync.dma_start(out=xT_t[:, 0:1024], in_=xT_d[:, 0:1024])
            nc.sync.dma_start(out=xT_t[:, 1024:2048], in_=xT_d[:, 1024:2048])
            return [[xT_t[:, (fh * HH + hh) * 512:(fh * HH + hh + 1) * 512]
                     for hh in range(HH)] for fh in range(FH)]

        def stage_b(xT_s):
            """x @ T into the c-interleaved M2 layout; returns the m2 tile."""
            m2 = m2p.tile([128, IPAD * HL * C], f16, tag="m2")
            m2v = m2.rearrange("p (x c) -> p x c", c=C)
            for g in range(4):
                ps = psB.tile([128, 1024], f32, tag="psB")
                for cl in range(2):
                    c = 2 * g + cl
                    for hh in range(HH):
                        for fh in range(FH):
                            nc.tensor.matmul(
                                ps[64 * hh:64 * hh + 64,
                                   cl * 512:(cl + 1) * 512],
                                lhsT=tw_s[fh][c], rhs=xT_s[fh][hh],
                                start=(fh == 0), stop=(fh == 1),
                                tile_position=(0, 64 * hh),
                                skip_group_check=True,
                            )
                # m2[x*8 + c] <- ps[(c2, x512)]
                nc.scalar.copy(
                    out=m2v[:, 0:512, 2 * g:2 * g + 2],
                    in_=ps.rearrange("p (c x) -> p x c", c=2))
            # circular pad: i in [32,48) := i in [0,16)  (ACT)
            nc.scalar.copy(out=m2[:, 4096:6144], in_=m2[:, 0:2048])
            return m2

        def dve_chunks(m2, chunks=DVE_CHUNKS, nrm=None):
            if nrm is None:
                nrm = nrmp.tile([128, 15 * 512 + 256], f16, tag="nrm")
            for d0, d1 in chunks:
                nd = min(d1, 15) - d0 + 1
                if d0 <= 15:
                    in0 = m2[:, 128 * d0:128 * d0 + 4096].unsqueeze(
                        1).to_broadcast([128, nd, 4096]).copy()
                    in0.ap[1] = (128, nd)    # [step, num]: one i per d
                    in1 = m2[:, 0:4096].unsqueeze(1).to_broadcast(
                        [128, nd, 4096])
                    o = nrm[:, (d0 - 1) * 512:(d0 - 1 + nd) * 512]
                    bi = nc.vector._custom_dve(
                        ps8, out=o.rearrange("p (d x) -> p d x", d=nd),
                        in0=in0, in1=in1)
                    bi.ins.perf_max = 1
                else:
                    bi = nc.vector._custom_dve(
                        ps8, out=nrm[:, 7680:7936],
                        in0=m2[:, 2048:4096], in1=m2[:, 0:2048])
                    bi.ins.perf_max = 1
            return nrm

        def exp_chunk(nrm, E, d0, d1):
            a = (d0 - 1) * 512
            b = min(d1, 15) * 512 if d1 < 16 else 7936
            nc.scalar.activation(
                out=E[:, a:b], in_=nrm[:, a:b],
                func=mybir.ActivationFunctionType.Exp, scale=-1.0,
            )

        def jsum_chunk(E, o_ps, d0, d1):
            for d in range(d0, d1 + 1):
                if d < 16:
                    Ed = E[:, (d - 1) * 512:d * 512]
                    sh = 16 * d
                    nc.tensor.matmul(o_ps[:, :], lhsT=iw_t, rhs=Ed,
                                     start=(d == 1), stop=False,
                                     skip_group_check=True)
                    nc.tensor.matmul(o_ps[:, sh:512], lhsT=iw_t,
                                     rhs=Ed[:, 0:512 - sh], start=False,
                                     stop=False, skip_group_check=True)
                    nc.tensor.matmul(o_ps[:, 0:sh], lhsT=iw_t,
                                     rhs=Ed[:, 512 - sh:512], start=False,
                                     stop=False, skip_group_check=True)
                else:
                    E16 = E[:, 7680:7936]
                    nc.tensor.matmul(o_ps[:, 0:256], lhsT=iw_t, rhs=E16,
                                     start=False, stop=False,
                                     skip_group_check=True)
                    nc.tensor.matmul(o_ps[:, 256:512], lhsT=iw_t, rhs=E16,
                                     start=False, stop=True,
                                     skip_group_check=True)

        def finish(nrm, chunks=CHUNKS):
            """exp + j-sum + drain + out-DMA for one body's norms."""
            E = Ep.tile([128, 15 * 512 + 256], f16, tag="E")
            o_ps = psO.tile([128, 512], f32, tag="oPs")
            for d0, d1 in chunks:
                exp_chunk(nrm, E, d0, d1)
                jsum_chunk(E, o_ps, d0, d1)
            o_sb = osb.tile([128, 512], f32, tag="osb")
            nc.scalar.activation(
                out=o_sb, in_=o_ps[:, :],
                func=mybir.ActivationFunctionType.Identity,
                bias=1.0, scale=1.0,
            )
            nc.sync.dma_start(out=o_d, in_=o_sb)

        # -- software-pipelined unrolled block --
        m2 = stage_b(load_x(0))
        nrm_prev = None
        for k in range(unroll):
            if k + 1 < unroll:
                nrm = dve_chunks(m2)
                m2 = stage_b(load_x(k + 1))
                if nrm_prev is not None:
                    finish(nrm_prev)
                nrm_prev = nrm
            else:
                # last body: fine DVE chunks with exp/j-sum trailing one
                # chunk so the post-barrier tail is only the final chunk.
                FINE = ((1, 5), (6, 10), (11, 15), (16, 16))
                nrm = nrmp.tile([128, 15 * 512 + 256], f16, tag="nrm")
                E = Ep.tile([128, 15 * 512 + 256], f16, tag="E")
                o_ps = psO.tile([128, 512], f32, tag="oPs")
                for ci, (d0, d1) in enumerate(FINE):
                    dve_chunks(m2, chunks=((d0, d1),), nrm=nrm)
                    if ci == 0 and nrm_prev is not None:
                        finish(nrm_prev)
                    if ci > 0:
                        pd0, pd1 = FINE[ci - 1]
                        exp_chunk(nrm, E, pd0, pd1)
                        jsum_chunk(E, o_ps, pd0, pd1)
                exp_chunk(nrm, E, *FINE[-1])
                jsum_chunk(E, o_ps, *FINE[-1])
                o_sb = osb.tile([128, 512], f32, tag="osb")
                nc.scalar.activation(
                    out=o_sb, in_=o_ps[:, :],
                    func=mybir.ActivationFunctionType.Identity,
                    bias=1.0, scale=1.0,
                )
                nc.sync.dma_start(out=o_d, in_=o_sb)


# --------------------------------------------------------------------------
# host side
# --------------------------------------------------------------------------

def prep_inputs(x, T):
    """Shared (core-independent) device inputs, packed partition-first."""
    xf = np.ascontiguousarray(x.reshape(N, HW, F))
    # tw[f, (fh, c, b)] = T[fh*128+f, b, c]
    tw = T.reshape(FH, 128, B, C).transpose(1, 0, 3, 2)     # f, fh, c, b
    tw_in = np.ascontiguousarray(tw.reshape(128, FH * C * B)).astype(np.float16)
    iw_in = np.eye(128, dtype=np.float16)
    return xf, tw_in, iw_in


def core_in_map(xf, tw_in, iw_in, k):
    xs = xf[:, k * HWL:(k + 1) * HWL, :]          # [i, hwl, f]
    # xT[f, (fh, hh, i, hl)] = x[i, hh*16+hl, fh*128+f]
    xT = xs.reshape(N, HH, HL, FH, 128).transpose(4, 3, 1, 0, 2)
    xT = np.ascontiguousarray(xT.reshape(128, FH * HH * N * HL))
    return {"xT": xT.astype(np.float16), "tw": tw_in, "iw": iw_in}


def gather_ob(core_outs):
    """core_outs: list of 8 arrays [128,512] f32 -> o_b [N,16,16,B]."""
    obs = []
    for res in core_outs:
        v = res.astype(np.float32).reshape(HH, B, N, HL)   # hh, b, i, hl
        obs.append(v.transpose(2, 0, 3, 1).reshape(N, HWL, B))  # i, hwl, b
    return np.concatenate(obs, axis=1).reshape(N, 16, 16, B)


def _get_program(reps=1, loop=None, unroll=UNROLL):
    key = ("nc", reps, loop, unroll)
    if key in _CACHED:
        return _CACHED[key]
    from contextlib import ExitStack
    import concourse.bacc as bacc
    import concourse.mybir as mybir
    import concourse.tile as tile

    _get_ps8_op()
    nc = bacc.Bacc("TRN2", target_bir_lowering=False, debug=False,
                   num_devices=CORES)
    f16, f32 = mybir.dt.float16, mybir.dt.float32
    ins = {
        "xT": nc.dram_tensor("xT", [128, FH * HH * N * HL], f16,
                             kind="ExternalInput").ap(),
        "tw": nc.dram_tensor("tw", [128, FH * C * B], f16,
                             kind="ExternalInput").ap(),
        "iw": nc.dram_tensor("iw", [128, 128], f16,
                             kind="ExternalInput").ap(),
    }
    outs = {
        "o": nc.dram_tensor("o", [128, 512], f32, kind="ExternalOutput").ap(),
    }
    with tile.TileContext(nc) as tc:
        if loop:
            with ExitStack() as ctx:
                pools = make_pools(tc, ctx)
                with tc.For_i(0, loop, 1,
                              hint_engines=(mybir.EngineType.PE,
                                            mybir.EngineType.DVE)):
                    build_block(tc, outs, ins, pools=pools, unroll=unroll)
        else:
            for r in range(reps):
                build_block(tc, outs, ins, rep=r, unroll=1)
    nc.compile()
    _CACHED[key] = nc
    return nc


def kernel(x, T):
    x = np.asarray(x, dtype=np.float32)
    T = np.asarray(T, dtype=np.float32)
    from concourse.bass_utils import run_bass_kernel_spmd

    nc = _get_program()
    xf, tw_in, iw_in = prep_inputs(x, T)
    in_maps = [core_in_map(xf, tw_in, iw_in, k) for k in range(CORES)]
    res = run_bass_kernel_spmd(nc, in_maps, core_ids=list(range(CORES)))
    ob = gather_ob([r["o"] for r in res.results])
    return np.concatenate([x, ob], axis=3)


# revision 20
# speedup vs baseline: 2.1849x; 1.0232x over previous
"""MiniBatchDiscrimination Trainium2 kernel — DVE-fused-norm version (v4).

reference:
    M = einsum('nhwf,fbc->nhwbc', x, T)          # [N,H,W,B,C]
    norm = sum_c |M[i] - M[j]|                   # [N,N,H,W,B]
    o_b  = sum_j exp(-norm)                      # [N,H,W,B]
    out  = concat([x, o_b], axis=3)              # [N,H,W,F+B]

Sharding: embarrassingly parallel over HW=256 spatial positions; each core
takes a 32-position hw slice and computes all pairs for it.

Core trick: the c-axis lives INNERMOST IN THE FREE DIM so a custom 8-state
DVE micro-op ("PAIRSUM8", 2X_1PORT) computes, per (i,hwl,b) position, the
full L1 norm sum_c |M2[i+d] - M2[i]| in one streaming pass — consuming
2 f16/lane/cycle and emitting one compacted f16 norm per 8 inputs.  This
removes the PE c-reduce matmuls entirely; PE only does the small x@T
projection (stage B) and identity-weight j-sum folds.

  M2   [128 part=(hh2,b64), 6144 free=(i48,hl16,c8)] f16; i in [32,48) is a
       circular pad (copy of i in [0,16)) so every d-stream is contiguous.
  nrm  [128, (d15,i32,hl16 | d16:i16,hl16)] f16 — d=1..15 in giant DVE
       instructions via a 3D AP (outer dim = d, stride one i; in1 broadcast).
  E    exp(-nrm) f16 (ACT)
  o_ps [128 part=(hh,b), 512=(i,hl)] f32 PSUM — j-sum via identity-weight
       matmuls: direct (col) + shifted (col+16d mod 512) accumulation;
       diagonal +1 fused into the drain.

Scheduling: tc.For_i puts an all-engine barrier between iterations, so the
loop body is U manually-unrolled executions, software-pipelined in trace
order: body k's DVE streams while PE/ACT build body k+1's M2 and compute
body k-1's exp/j-sum/drain.  Steady state is DVE-bound.
"""

import os
import sys

for _p in ("/opt/trn_rl_repo", "/opt/pypackages"):
    if _p not in sys.path and os.path.isdir(_p):
        sys.path.append(_p)

import numpy as np

N, HWL, F, B, C = 32, 32, 256, 64, 8
HW = 256
CORES = 8
FH = 2           # f in two partition halves of 128
HH = 2           # hwl_hi: hw position high bit (partition dim)
HL = 16          # hwl_lo: 16 positions (free dim)
IPAD = 48        # i padded 32 -> 48 for circular d-shift reads
UNROLL = 24      # bodies per For_i iteration (amortizes the loop barrier)

_CACHED = {}


# --------------------------------------------------------------------------
# PAIRSUM8 custom DVE op: out[g] = sum_{k<8} |in0[8g+k] - in1[8g+k]|
# 2X_1PORT 8-state machine; emits two group sums (WR0_LO/WR0_HI) every
# 4th pair-cycle; out FD = in FD / 8, contiguous.
# --------------------------------------------------------------------------

def _mk_ps8_2x():
    from concourse.dve_uop import (
        ENABLE, AluInp, AluOp, DelayInp, InpSel, OutPath, OutSel, Trigger,
        UopConfig, UopDpConfig,
    )

    def base(next_idx):
        u = UopConfig()
        u.enable_input(InpSel.SRC_0, 0).enable_input(InpSel.SRC_1, 1)
        u.enable_input(InpSel.SRC_0_HI, 2).enable_input(InpSel.SRC_1_HI, 3)
        u.enable_input(InpSel.ZERO, 4)
        u.require_inp0 = ENABLE
        u.require_inp1 = ENABLE
        u.trigger = (Trigger.SRC_TENSOR_DONE, Trigger.COUNT, Trigger.NONE)
        u.next_uop = (0, next_idx, 0)
        u.repeat_count = 1
        dp = u.datapath_config
        # s0: |a_lo-b_lo|; carry a_hi(d1), b_hi(d2), zero(d3)
        dp[0] = (UopDpConfig()
                 .enable_alu(AluOp.ABSOLUTE_DIFF, AluInp.PREV_ALU_OUT,
                             AluInp.PREV_DELAY_0)
                 .pass_through_delay(1, 2, 3))
        # s1: |a_hi-b_hi|; d0 <- lo result
        dp[1] = (UopDpConfig()
                 .enable_alu(AluOp.ABSOLUTE_DIFF, AluInp.PREV_DELAY_1,
                             AluInp.PREV_DELAY_2)
                 .enable_delay_from_src(DelayInp.PREV_ALU_OUT, 0)
                 .pass_through_delay(3))
        # s2: pair sum s = lo + hi
        dp[2] = (UopDpConfig()
                 .enable_alu(AluOp.ADD, AluInp.PREV_ALU_OUT,
                             AluInp.PREV_DELAY_0)
                 .pass_through_delay(3))
        for i in range(3, 8):
            dp[i] = UopDpConfig().pass_through_alu().pass_through_delay(3)
        return u

    def s0(nx):            # park flopA = s + 0
        u = base(nx)
        u.datapath_config[3] = (UopDpConfig()
                                .enable_alu(AluOp.ADD, AluInp.PREV_ALU_OUT,
                                            AluInp.PREV_DELAY_3)
                                .pass_through_delay(3))
        return u

    def sa(nx):            # flopA += s
        u = base(nx)
        u.datapath_config[3] = (UopDpConfig()
                                .enable_alu(AluOp.ADD, AluInp.PREV_ALU_OUT,
                                            AluInp.CURR_ALU_OUT)
                                .pass_through_delay(3))
        return u

    def s4(nx):            # s3 off (s rides d1); flopB = s + 0
        u = base(nx)
        u.datapath_config[3] = (UopDpConfig()
                                .enable_delay_from_src(DelayInp.PREV_ALU_OUT, 1)
                                .pass_through_delay(3))
        u.datapath_config[4] = (UopDpConfig()
                                .enable_alu(AluOp.ADD, AluInp.PREV_DELAY_1,
                                            AluInp.PREV_DELAY_3)
                                .pass_through_delay(3))
        return u

    def sb(nx):            # flopB += s
        u = base(nx)
        u.datapath_config[3] = (UopDpConfig()
                                .enable_delay_from_src(DelayInp.PREV_ALU_OUT, 1)
                                .pass_through_delay(3))
        u.datapath_config[4] = (UopDpConfig()
                                .enable_alu(AluOp.ADD, AluInp.PREV_DELAY_1,
                                            AluInp.CURR_ALU_OUT)
                                .pass_through_delay(3))
        return u

    def s7(nx):            # emit q0 (flopA export via d2) + q1 (alu)
        u = base(nx)
        u.datapath_config[3] = (UopDpConfig()
                                .enable_alu(AluOp.ADD, AluInp.PREV_DELAY_3,
                                            AluInp.CURR_ALU_OUT)
                                .enable_delay_from_src(DelayInp.PREV_ALU_OUT, 1)
                                .pass_through_delay(3))
        u.datapath_config[4] = (UopDpConfig()
                                .enable_alu(AluOp.ADD, AluInp.PREV_DELAY_1,
                                            AluInp.CURR_ALU_OUT)
                                .enable_delay_from_src(DelayInp.PREV_ALU_OUT, 2)
                                .pass_through_delay(3))
        for i in range(5, 8):
            u.datapath_config[i] = (UopDpConfig().pass_through_alu()
                                    .pass_through_delay(2, 3))
        u.enable_output(OutSel.DELAY_2, OutPath.WR0_LO)
        u.enable_output(OutSel.ALU_OUT, OutPath.WR0_HI)
        return u

    # idx: 0=S0entry 1..7=S1..S7 8=S0loop (0 = IDLE, loop restarts at 8)
    return [s0(1), sa(2), sa(3), sa(4), s4(5), sb(6), sb(7), s7(8), s0(1)]


def _get_ps8_op():
    if "ps8" in _CACHED:
        return _CACHED["ps8"]
    from concourse import dve_ops
    from concourse.dve_spec import Spec, Src0, Src1, maxx
    from concourse.dve_uop import DveOpSpec

    NAME = "PAIRSUM8_ANT"
    for op in dve_ops.OPS:
        if op.name == NAME:
            _CACHED["ps8"] = op
            return op
    spec = Spec(
        body=maxx(Src0 - Src1, Src1 - Src0),
        reference=lambda in0, in1, s0, s1, imm2: np.abs(
            in0.astype(np.float32) - in1.astype(np.float32)),
    )
    op = dve_ops.DveOp(NAME, spec, subdim=False, uops_sha={})
    dve_ops.OPS.append(op)
    dve_ops.CUSTOM_DVE_SPECS[op.name] = op.spec
    row = dve_ops._CUSTOM_DVE_ROW_BASE + len(dve_ops.OPS) - 1
    dve_ops._SUB_OPCODE_FOR_NAME[op.name] = row
    uops = _mk_ps8_2x()
    compiled = DveOpSpec(
        name=NAME, opcode=row, uops=uops, uops_2x=uops,
        perf_max=1, rd1_en=True,
    )
    compiled.validate("v3")
    dve_ops._COMPILE_CACHE[(NAME, "v3")] = compiled
    dve_ops._COMPILE_CACHE[(NAME, "v4")] = compiled
    _CACHED["ps8"] = op
    return op


# --------------------------------------------------------------------------
# device program
# --------------------------------------------------------------------------

def make_pools(tc, ctx, rep=0):
    sfx = f"_{rep}"
    singles = ctx.enter_context(tc.tile_pool(name="singles" + sfx, bufs=1))
    xTp = ctx.enter_context(tc.tile_pool(name="xTp" + sfx, bufs=2))
    m2p = ctx.enter_context(tc.tile_pool(name="m2p" + sfx, bufs=2))
    psB = ctx.enter_context(tc.tile_pool(name="psB" + sfx, bufs=3,
                                         space="PSUM"))
    nrmp = ctx.enter_context(tc.tile_pool(name="nrmp" + sfx, bufs=2))
    Ep = ctx.enter_context(tc.tile_pool(name="Ep" + sfx, bufs=2))
    psO = ctx.enter_context(tc.tile_pool(name="psO" + sfx, bufs=2,
                                         space="PSUM"))
    osb = ctx.enter_context(tc.tile_pool(name="osb" + sfx, bufs=2))
    return singles, xTp, m2p, psB, nrmp, Ep, psO, osb


# d-stream chunks: (d_first, d_last); 16 handled as the half-range stream
CHUNKS = ((1, 8), (9, 15), (16, 16))       # exp/j-sum granularity
DVE_CHUNKS = ((1, 15), (16, 16))           # DVE instruction granularity


def build_block(tc, outs, ins, rep=0, pools=None, unroll=1):
    """Trace `unroll` software-pipelined executions of the kernel body.

    ins:  xT  [128,2048] f16  xT[f,(fh,hh,i,hl)] = x[i, hw(core,hh,hl), fh*128+f]
          tw  [128,1024] f16  tw[f,(fh,c,b)] = T[fh*128+f, b, c]
          iw  [128,128]  f16  identity
    outs: o   [128,512]  f32  o[hh*64+b, i*16+hl] = o_b[i, hw(core,hh,hl), b]
    """
    from contextlib import ExitStack

    import concourse.mybir as mybir

    nc = tc.nc
    f16 = mybir.dt.float16
    f32 = mybir.dt.float32
    ps8 = _get_ps8_op()

    xT_d, tw_d, iw_d = ins["xT"], ins["tw"], ins["iw"]
    o_d = outs["o"]

    with ExitStack() as ctx:
        if pools is None:
            pools = make_pools(tc, ctx, rep)
        singles, xTp, m2p, psB, nrmp, Ep, psO, osb = pools

        tw_t = singles.tile([128, FH * C * B], f16, tag="tw")
        nc.sync.dma_start(out=tw_t, in_=tw_d)
        iw_t = singles.tile([128, 128], f16, tag="iw")
        nc.sync.dma_start(out=iw_t, in_=iw_d)
        tw_s = [[tw_t[:, (fh * C + c) * B:(fh * C + c + 1) * B]
                 for c in range(C)] for fh in range(FH)]

        def load_x(k):
            xT_t = xTp.tile([128, FH * HH * N * HL], f16, tag="xT")
            nc.sync.dma_start(out=xT_t[:, 0:1024], in_=xT_d[:, 0:1024])
            nc.sync.dma_start(out=xT_t[:, 1024:2048], in_=xT_d[:, 1024:2048])
            return [[xT_t[:, (fh * HH + hh) * 512:(fh * HH + hh + 1) * 512]
                     for hh in range(HH)] for fh in range(FH)]

        def stage_b(xT_s):
            """x @ T into the c-interleaved M2 layout; returns the m2 tile."""
            m2 = m2p.tile([128, IPAD * HL * C], f16, tag="m2")
            m2v = m2.rearrange("p (x c) -> p x c", c=C)
            for g in range(4):
                ps = psB.tile([128, 1024], f32, tag="psB")
                for cl in range(2):
                    c = 2 * g + cl
                    for hh in range(HH):
                        for fh in range(FH):
                            nc.tensor.matmul(
                                ps[64 * hh:64 * hh + 64,
                                   cl * 512:(cl + 1) * 512],
                                lhsT=tw_s[fh][c], rhs=xT_s[fh][hh],
                                start=(fh == 0), stop=(fh == 1),
                                tile_position=(0, 64 * hh),
                                skip_group_check=True,
                            )
                # m2[x*8 + c] <- ps[(c2, x512)]
                nc.scalar.copy(
                    out=m2v[:, 0:512, 2 * g:2 * g + 2],
                    in_=ps.rearrange("p (c x) -> p x c", c=2))
            # circular pad: i in [32,48) := i in [0,16)  (ACT)
            nc.scalar.copy(out=m2[:, 4096:6144], in_=m2[:, 0:2048])
            return m2

        def dve_chunks(m2, chunks=DVE_CHUNKS, nrm=None):
            if nrm is None:
                nrm = nrmp.tile([128, 15 * 512 + 256], f16, tag="nrm")
            for d0, d1 in chunks:
                nd = min(d1, 15) - d0 + 1
                if d0 <= 15:
                    in0 = m2[:, 128 * d0:128 * d0 + 4096].unsqueeze(
                        1).to_broadcast([128, nd, 4096]).copy()
                    in0.ap[1] = (128, nd)    # [step, num]: one i per d
                    in1 = m2[:, 0:4096].unsqueeze(1).to_broadcast(
                        [128, nd, 4096])
                    o = nrm[:, (d0 - 1) * 512:(d0 - 1 + nd) * 512]
                    bi = nc.vector._custom_dve(
                        ps8, out=o.rearrange("p (d x) -> p d x", d=nd),
                        in0=in0, in1=in1)
                    bi.ins.perf_max = 1
                else:
                    bi = nc.vector._custom_dve(
                        ps8, out=nrm[:, 7680:7936],
                        in0=m2[:, 2048:4096], in1=m2[:, 0:2048])
                    bi.ins.perf_max = 1
            return nrm

        def exp_chunk(nrm, E, d0, d1):
            a = (d0 - 1) * 512
            b = min(d1, 15) * 512 if d1 < 16 else 7936
            nc.scalar.activation(
                out=E[:, a:b], in_=nrm[:, a:b],
                func=mybir.ActivationFunctionType.Exp, scale=-1.0,
            )

        def jsum_chunk(E, o_ps, d0, d1):
            for d in range(d0, d1 + 1):
                if d < 16:
                    Ed = E[:, (d - 1) * 512:d * 512]
                    sh = 16 * d
                    nc.tensor.matmul(o_ps[:, :], lhsT=iw_t, rhs=Ed,
                                     start=(d == 1), stop=False,
                                     skip_group_check=True)
                    nc.tensor.matmul(o_ps[:, sh:512], lhsT=iw_t,
                                     rhs=Ed[:, 0:512 - sh], start=False,
                                     stop=False, skip_group_check=True)
                    nc.tensor.matmul(o_ps[:, 0:sh], lhsT=iw_t,
                                     rhs=Ed[:, 512 - sh:512], start=False,
                                     stop=False, skip_group_check=True)
                else:
                    E16 = E[:, 7680:7936]
                    nc.tensor.matmul(o_ps[:, 0:256], lhsT=iw_t, rhs=E16,
                                     start=False, stop=False,
                                     skip_group_check=True)
                    nc.tensor.matmul(o_ps[:, 256:512], lhsT=iw_t, rhs=E16,
                                     start=False, stop=True,
                                     skip_group_check=True)

        def finish(nrm, chunks=CHUNKS):
            """exp + j-sum + drain + out-DMA for one body's norms."""
            E = Ep.tile([128, 15 * 512 + 256], f16, tag="E")
            o_ps = psO.tile([128, 512], f32, tag="oPs")
            for d0, d1 in chunks:
                exp_chunk(nrm, E, d0, d1)
                jsum_chunk(E, o_ps, d0, d1)
            o_sb = osb.tile([128, 512], f32, tag="osb")
            nc.scalar.activation(
                out=o_sb, in_=o_ps[:, :],
                func=mybir.ActivationFunctionType.Identity,
                bias=1.0, scale=1.0,
            )
            nc.sync.dma_start(out=o_d, in_=o_sb)

        # -- software-pipelined unrolled block --
        m2 = stage_b(load_x(0))
        nrm_prev = None
        for k in range(unroll):
            if k + 1 < unroll:
                nrm = dve_chunks(m2)
                m2 = stage_b(load_x(k + 1))
                if nrm_prev is not None:
                    finish(nrm_prev)
                nrm_prev = nrm
            else:
                # last body: fine DVE chunks with exp/j-sum trailing one
                # chunk so the post-barrier tail is only the final chunk.
                FINE = ((1, 5), (6, 10), (11, 15), (16, 16))
                nrm = nrmp.tile([128, 15 * 512 + 256], f16, tag="nrm")
                E = Ep.tile([128, 15 * 512 + 256], f16, tag="E")
                o_ps = psO.tile([128, 512], f32, tag="oPs")
                for ci, (d0, d1) in enumerate(FINE):
                    dve_chunks(m2, chunks=((d0, d1),), nrm=nrm)
                    if ci == 0 and nrm_prev is not None:
                        finish(nrm_prev)
                    if ci > 0:
                        pd0, pd1 = FINE[ci - 1]
                        exp_chunk(nrm, E, pd0, pd1)
                        jsum_chunk(E, o_ps, pd0, pd1)
                exp_chunk(nrm, E, *FINE[-1])
                jsum_chunk(E, o_ps, *FINE[-1])
                o_sb = osb.tile([128, 512], f32, tag="osb")
                nc.scalar.activation(
                    out=o_sb, in_=o_ps[:, :],
                    func=mybir.ActivationFunctionType.Identity,
                    bias=1.0, scale=1.0,
                )
                nc.sync.dma_start(out=o_d, in_=o_sb)


# --------------------------------------------------------------------------
# host side
# --------------------------------------------------------------------------

def prep_inputs(x, T):
    """Shared (core-independent) device inputs, packed partition-first."""
    xf = np.ascontiguousarray(x.reshape(N, HW, F))
    # tw[f, (fh, c, b)] = T[fh*128+f, b, c]
    tw = T.reshape(FH, 128, B, C).transpose(1, 0, 3, 2)     # f, fh, c, b
    tw_in = np.ascontiguousarray(tw.reshape(128, FH * C * B)).astype(np.float16)
    iw_in = np.eye(128, dtype=np.float16)
    return xf, tw_in, iw_in


def core_in_map(xf, tw_in, iw_in, k):
    xs = xf[:, k * HWL:(k + 1) * HWL, :]          # [i, hwl, f]
    # xT[f, (fh, hh, i, hl)] = x[i, hh*16+hl, fh*128+f]
    xT = xs.reshape(N, HH, HL, FH, 128).transpose(4, 3, 1, 0, 2)
    xT = np.ascontiguousarray(xT.reshape(128, FH * HH * N * HL))
    return {"xT": xT.astype(np.float16), "tw": tw_in, "iw": iw_in}


def gather_ob(core_outs):
    """core_outs: list of 8 arrays [128,512] f32 -> o_b [N,16,16,B]."""
    obs = []
    for res in core_outs:
        v = res.astype(np.float32).reshape(HH, B, N, HL)   # hh, b, i, hl
        obs.append(v.transpose(2, 0, 3, 1).reshape(N, HWL, B))  # i, hwl, b
    return np.concatenate(obs, axis=1).reshape(N, 16, 16, B)


def _get_program(reps=1, loop=None, unroll=UNROLL):
    key = ("nc", reps, loop, unroll)
    if key in _CACHED:
        return _CACHED[key]
    from contextlib import ExitStack
    import concourse.bacc as bacc
    import concourse.mybir as mybir
    import concourse.tile as tile

    _get_ps8_op()
    nc = bacc.Bacc("TRN2", target_bir_lowering=False, debug=False,
                   num_devices=CORES)
    f16, f32 = mybir.dt.float16, mybir.dt.float32
    ins = {
        "xT": nc.dram_tensor("xT", [128, FH * HH * N * HL], f16,
                             kind="ExternalInput").ap(),
        "tw": nc.dram_tensor("tw", [128, FH * C * B], f16,
                             kind="ExternalInput").ap(),
        "iw": nc.dram_tensor("iw", [128, 128], f16,
                             kind="ExternalInput").ap(),
    }
    outs = {
        "o": nc.dram_tensor("o", [128, 512], f32, kind="ExternalOutput").ap(),
    }
    with tile.TileContext(nc) as tc:
        if loop:
            with ExitStack() as ctx:
                pools = make_pools(tc, ctx)
                with tc.For_i(0, loop, 1,
                              hint_engines=(mybir.EngineType.PE,
                                            mybir.EngineType.DVE)):
                    build_block(tc, outs, ins, pools=pools, unroll=unroll)
        else:
            for r in range(reps):
                build_block(tc, outs, ins, rep=r, unroll=1)
    nc.compile()
    _CACHED[key] = nc
    return nc


def kernel(x, T):
    x = np.asarray(x, dtype=np.float32)
    T = np.asarray(T, dtype=np.float32)
    from concourse.bass_utils import run_bass_kernel_spmd

    nc = _get_program()
    xf, tw_in, iw_in = prep_inputs(x, T)
    in_maps = [core_in_map(xf, tw_in, iw_in, k) for k in range(CORES)]
    res = run_bass_kernel_spmd(nc, in_maps, core_ids=list(range(CORES)))
    ob = gather_ob([r["o"] for r in res.results])
    return np.concatenate([x, ob], axis=3)


# revision 21
# speedup vs baseline: 2.2110x; 1.0120x over previous
"""MiniBatchDiscrimination Trainium2 kernel — DVE-fused-norm version (v4).

reference:
    M = einsum('nhwf,fbc->nhwbc', x, T)          # [N,H,W,B,C]
    norm = sum_c |M[i] - M[j]|                   # [N,N,H,W,B]
    o_b  = sum_j exp(-norm)                      # [N,H,W,B]
    out  = concat([x, o_b], axis=3)              # [N,H,W,F+B]

Sharding: embarrassingly parallel over HW=256 spatial positions; each core
takes a 32-position hw slice and computes all pairs for it.

Core trick: the c-axis lives INNERMOST IN THE FREE DIM so a custom 8-state
DVE micro-op ("PAIRSUM8", 2X_1PORT) computes, per (i,hwl,b) position, the
full L1 norm sum_c |M2[i+d] - M2[i]| in one streaming pass — consuming
2 f16/lane/cycle and emitting one compacted f16 norm per 8 inputs.  This
removes the PE c-reduce matmuls entirely; PE only does the small x@T
projection (stage B) and identity-weight j-sum folds.

  M2   [128 part=(hh2,b64), 6144 free=(i48,hl16,c8)] f16; i in [32,48) is a
       circular pad (copy of i in [0,16)) so every d-stream is contiguous.
  nrm  [128, (d15,i32,hl16 | d16:i16,hl16)] f16 — d=1..15 in giant DVE
       instructions via a 3D AP (outer dim = d, stride one i; in1 broadcast).
  E    exp(-nrm) f16 (ACT)
  o_ps [128 part=(hh,b), 512=(i,hl)] f32 PSUM — j-sum via identity-weight
       matmuls: direct (col) + shifted (col+16d mod 512) accumulation;
       diagonal +1 fused into the drain.

Scheduling: tc.For_i puts an all-engine barrier between iterations, so the
loop body is U manually-unrolled executions, software-pipelined in trace
order: body k's DVE streams while PE/ACT build body k+1's M2 and compute
body k-1's exp/j-sum/drain.  Steady state is DVE-bound.
"""

import os
import sys

for _p in ("/opt/trn_rl_repo", "/opt/pypackages"):
    if _p not in sys.path and os.path.isdir(_p):
        sys.path.append(_p)

import numpy as np

N, HWL, F, B, C = 32, 32, 256, 64, 8
HW = 256
CORES = 8
FH = 2           # f in two partition halves of 128
HH = 2           # hwl_hi: hw position high bit (partition dim)
HL = 16          # hwl_lo: 16 positions (free dim)
IPAD = 48        # i padded 32 -> 48 for circular d-shift reads
UNROLL = 32      # bodies per For_i iteration (amortizes the loop barrier)

_CACHED = {}


# --------------------------------------------------------------------------
# PAIRSUM8 custom DVE op: out[g] = sum_{k<8} |in0[8g+k] - in1[8g+k]|
# 2X_1PORT 8-state machine; emits two group sums (WR0_LO/WR0_HI) every
# 4th pair-cycle; out FD = in FD / 8, contiguous.
# --------------------------------------------------------------------------

def _mk_ps8_2x():
    from concourse.dve_uop import (
        ENABLE, AluInp, AluOp, DelayInp, InpSel, OutPath, OutSel, Trigger,
        UopConfig, UopDpConfig,
    )

    def base(next_idx):
        u = UopConfig()
        u.enable_input(InpSel.SRC_0, 0).enable_input(InpSel.SRC_1, 1)
        u.enable_input(InpSel.SRC_0_HI, 2).enable_input(InpSel.SRC_1_HI, 3)
        u.enable_input(InpSel.ZERO, 4)
        u.require_inp0 = ENABLE
        u.require_inp1 = ENABLE
        u.trigger = (Trigger.SRC_TENSOR_DONE, Trigger.COUNT, Trigger.NONE)
        u.next_uop = (0, next_idx, 0)
        u.repeat_count = 1
        dp = u.datapath_config
        # s0: |a_lo-b_lo|; carry a_hi(d1), b_hi(d2), zero(d3)
        dp[0] = (UopDpConfig()
                 .enable_alu(AluOp.ABSOLUTE_DIFF, AluInp.PREV_ALU_OUT,
                             AluInp.PREV_DELAY_0)
                 .pass_through_delay(1, 2, 3))
        # s1: |a_hi-b_hi|; d0 <- lo result
        dp[1] = (UopDpConfig()
                 .enable_alu(AluOp.ABSOLUTE_DIFF, AluInp.PREV_DELAY_1,
                             AluInp.PREV_DELAY_2)
                 .enable_delay_from_src(DelayInp.PREV_ALU_OUT, 0)
                 .pass_through_delay(3))
        # s2: pair sum s = lo + hi
        dp[2] = (UopDpConfig()
                 .enable_alu(AluOp.ADD, AluInp.PREV_ALU_OUT,
                             AluInp.PREV_DELAY_0)
                 .pass_through_delay(3))
        for i in range(3, 8):
            dp[i] = UopDpConfig().pass_through_alu().pass_through_delay(3)
        return u

    def s0(nx):            # park flopA = s + 0
        u = base(nx)
        u.datapath_config[3] = (UopDpConfig()
                                .enable_alu(AluOp.ADD, AluInp.PREV_ALU_OUT,
                                            AluInp.PREV_DELAY_3)
                                .pass_through_delay(3))
        return u

    def sa(nx):            # flopA += s
        u = base(nx)
        u.datapath_config[3] = (UopDpConfig()
                                .enable_alu(AluOp.ADD, AluInp.PREV_ALU_OUT,
                                            AluInp.CURR_ALU_OUT)
                                .pass_through_delay(3))
        return u

    def s4(nx):            # s3 off (s rides d1); flopB = s + 0
        u = base(nx)
        u.datapath_config[3] = (UopDpConfig()
                                .enable_delay_from_src(DelayInp.PREV_ALU_OUT, 1)
                                .pass_through_delay(3))
        u.datapath_config[4] = (UopDpConfig()
                                .enable_alu(AluOp.ADD, AluInp.PREV_DELAY_1,
                                            AluInp.PREV_DELAY_3)
                                .pass_through_delay(3))
        return u

    def sb(nx):            # flopB += s
        u = base(nx)
        u.datapath_config[3] = (UopDpConfig()
                                .enable_delay_from_src(DelayInp.PREV_ALU_OUT, 1)
                                .pass_through_delay(3))
        u.datapath_config[4] = (UopDpConfig()
                                .enable_alu(AluOp.ADD, AluInp.PREV_DELAY_1,
                                            AluInp.CURR_ALU_OUT)
                                .pass_through_delay(3))
        return u

    def s7(nx):            # emit q0 (flopA export via d2) + q1 (alu)
        u = base(nx)
        u.datapath_config[3] = (UopDpConfig()
                                .enable_alu(AluOp.ADD, AluInp.PREV_DELAY_3,
                                            AluInp.CURR_ALU_OUT)
                                .enable_delay_from_src(DelayInp.PREV_ALU_OUT, 1)
                                .pass_through_delay(3))
        u.datapath_config[4] = (UopDpConfig()
                                .enable_alu(AluOp.ADD, AluInp.PREV_DELAY_1,
                                            AluInp.CURR_ALU_OUT)
                                .enable_delay_from_src(DelayInp.PREV_ALU_OUT, 2)
                                .pass_through_delay(3))
        for i in range(5, 8):
            u.datapath_config[i] = (UopDpConfig().pass_through_alu()
                                    .pass_through_delay(2, 3))
        u.enable_output(OutSel.DELAY_2, OutPath.WR0_LO)
        u.enable_output(OutSel.ALU_OUT, OutPath.WR0_HI)
        return u

    # idx: 0=S0entry 1..7=S1..S7 8=S0loop (0 = IDLE, loop restarts at 8)
    return [s0(1), sa(2), sa(3), sa(4), s4(5), sb(6), sb(7), s7(8), s0(1)]


def _get_ps8_op():
    if "ps8" in _CACHED:
        return _CACHED["ps8"]
    from concourse import dve_ops
    from concourse.dve_spec import Spec, Src0, Src1, maxx
    from concourse.dve_uop import DveOpSpec

    NAME = "PAIRSUM8_ANT"
    for op in dve_ops.OPS:
        if op.name == NAME:
            _CACHED["ps8"] = op
            return op
    spec = Spec(
        body=maxx(Src0 - Src1, Src1 - Src0),
        reference=lambda in0, in1, s0, s1, imm2: np.abs(
            in0.astype(np.float32) - in1.astype(np.float32)),
    )
    op = dve_ops.DveOp(NAME, spec, subdim=False, uops_sha={})
    dve_ops.OPS.append(op)
    dve_ops.CUSTOM_DVE_SPECS[op.name] = op.spec
    row = dve_ops._CUSTOM_DVE_ROW_BASE + len(dve_ops.OPS) - 1
    dve_ops._SUB_OPCODE_FOR_NAME[op.name] = row
    uops = _mk_ps8_2x()
    compiled = DveOpSpec(
        name=NAME, opcode=row, uops=uops, uops_2x=uops,
        perf_max=1, rd1_en=True,
    )
    compiled.validate("v3")
    dve_ops._COMPILE_CACHE[(NAME, "v3")] = compiled
    dve_ops._COMPILE_CACHE[(NAME, "v4")] = compiled
    _CACHED["ps8"] = op
    return op


# --------------------------------------------------------------------------
# device program
# --------------------------------------------------------------------------

def make_pools(tc, ctx, rep=0):
    sfx = f"_{rep}"
    singles = ctx.enter_context(tc.tile_pool(name="singles" + sfx, bufs=1))
    xTp = ctx.enter_context(tc.tile_pool(name="xTp" + sfx, bufs=2))
    m2p = ctx.enter_context(tc.tile_pool(name="m2p" + sfx, bufs=2))
    psB = ctx.enter_context(tc.tile_pool(name="psB" + sfx, bufs=3,
                                         space="PSUM"))
    nrmp = ctx.enter_context(tc.tile_pool(name="nrmp" + sfx, bufs=2))
    Ep = ctx.enter_context(tc.tile_pool(name="Ep" + sfx, bufs=2))
    psO = ctx.enter_context(tc.tile_pool(name="psO" + sfx, bufs=2,
                                         space="PSUM"))
    osb = ctx.enter_context(tc.tile_pool(name="osb" + sfx, bufs=2))
    return singles, xTp, m2p, psB, nrmp, Ep, psO, osb


# d-stream chunks: (d_first, d_last); 16 handled as the half-range stream
CHUNKS = ((1, 8), (9, 15), (16, 16))       # exp/j-sum granularity
DVE_CHUNKS = ((1, 15), (16, 16))           # DVE instruction granularity


def build_block(tc, outs, ins, rep=0, pools=None, unroll=1):
    """Trace `unroll` software-pipelined executions of the kernel body.

    ins:  xT  [128,2048] f16  xT[f,(fh,hh,i,hl)] = x[i, hw(core,hh,hl), fh*128+f]
          tw  [128,1024] f16  tw[f,(fh,c,b)] = T[fh*128+f, b, c]
          iw  [128,128]  f16  identity
    outs: o   [128,512]  f32  o[hh*64+b, i*16+hl] = o_b[i, hw(core,hh,hl), b]
    """
    from contextlib import ExitStack

    import concourse.mybir as mybir

    nc = tc.nc
    f16 = mybir.dt.float16
    f32 = mybir.dt.float32
    ps8 = _get_ps8_op()

    xT_d, tw_d, iw_d = ins["xT"], ins["tw"], ins["iw"]
    o_d = outs["o"]

    with ExitStack() as ctx:
        if pools is None:
            pools = make_pools(tc, ctx, rep)
        singles, xTp, m2p, psB, nrmp, Ep, psO, osb = pools

        tw_t = singles.tile([128, FH * C * B], f16, tag="tw")
        nc.sync.dma_start(out=tw_t, in_=tw_d)
        iw_t = singles.tile([128, 128], f16, tag="iw")
        nc.sync.dma_start(out=iw_t, in_=iw_d)
        tw_s = [[tw_t[:, (fh * C + c) * B:(fh * C + c + 1) * B]
                 for c in range(C)] for fh in range(FH)]

        def load_x(k):
            xT_t = xTp.tile([128, FH * HH * N * HL], f16, tag="xT")
            nc.sync.dma_start(out=xT_t[:, 0:1024], in_=xT_d[:, 0:1024])
            nc.sync.dma_start(out=xT_t[:, 1024:2048], in_=xT_d[:, 1024:2048])
            return [[xT_t[:, (fh * HH + hh) * 512:(fh * HH + hh + 1) * 512]
                     for hh in range(HH)] for fh in range(FH)]

        def stage_b(xT_s):
            """x @ T into the c-interleaved M2 layout; returns the m2 tile."""
            m2 = m2p.tile([128, IPAD * HL * C], f16, tag="m2")
            m2v = m2.rearrange("p (x c) -> p x c", c=C)
            for g in range(4):
                ps = psB.tile([128, 1024], f32, tag="psB")
                for cl in range(2):
                    c = 2 * g + cl
                    for hh in range(HH):
                        for fh in range(FH):
                            nc.tensor.matmul(
                                ps[64 * hh:64 * hh + 64,
                                   cl * 512:(cl + 1) * 512],
                                lhsT=tw_s[fh][c], rhs=xT_s[fh][hh],
                                start=(fh == 0), stop=(fh == 1),
                                tile_position=(0, 64 * hh),
                                skip_group_check=True,
                            )
                # m2[x*8 + c] <- ps[(c2, x512)]
                nc.scalar.copy(
                    out=m2v[:, 0:512, 2 * g:2 * g + 2],
                    in_=ps.rearrange("p (c x) -> p x c", c=2))
            # circular pad: i in [32,48) := i in [0,16)  (ACT)
            nc.scalar.copy(out=m2[:, 4096:6144], in_=m2[:, 0:2048])
            return m2

        def dve_chunks(m2, chunks=DVE_CHUNKS, nrm=None):
            if nrm is None:
                nrm = nrmp.tile([128, 15 * 512 + 256], f16, tag="nrm")
            for d0, d1 in chunks:
                nd = min(d1, 15) - d0 + 1
                if d0 <= 15:
                    in0 = m2[:, 128 * d0:128 * d0 + 4096].unsqueeze(
                        1).to_broadcast([128, nd, 4096]).copy()
                    in0.ap[1] = (128, nd)    # [step, num]: one i per d
                    in1 = m2[:, 0:4096].unsqueeze(1).to_broadcast(
                        [128, nd, 4096])
                    o = nrm[:, (d0 - 1) * 512:(d0 - 1 + nd) * 512]
                    bi = nc.vector._custom_dve(
                        ps8, out=o.rearrange("p (d x) -> p d x", d=nd),
                        in0=in0, in1=in1)
                    bi.ins.perf_max = 1
                else:
                    bi = nc.vector._custom_dve(
                        ps8, out=nrm[:, 7680:7936],
                        in0=m2[:, 2048:4096], in1=m2[:, 0:2048])
                    bi.ins.perf_max = 1
            return nrm

        def exp_chunk(nrm, E, d0, d1):
            a = (d0 - 1) * 512
            b = min(d1, 15) * 512 if d1 < 16 else 7936
            nc.scalar.activation(
                out=E[:, a:b], in_=nrm[:, a:b],
                func=mybir.ActivationFunctionType.Exp, scale=-1.0,
            )

        def jsum_chunk(E, o_ps, d0, d1):
            for d in range(d0, d1 + 1):
                if d < 16:
                    Ed = E[:, (d - 1) * 512:d * 512]
                    sh = 16 * d
                    nc.tensor.matmul(o_ps[:, :], lhsT=iw_t, rhs=Ed,
                                     start=(d == 1), stop=False,
                                     skip_group_check=True)
                    nc.tensor.matmul(o_ps[:, sh:512], lhsT=iw_t,
                                     rhs=Ed[:, 0:512 - sh], start=False,
                                     stop=False, skip_group_check=True)
                    nc.tensor.matmul(o_ps[:, 0:sh], lhsT=iw_t,
                                     rhs=Ed[:, 512 - sh:512], start=False,
                                     stop=False, skip_group_check=True)
                else:
                    E16 = E[:, 7680:7936]
                    nc.tensor.matmul(o_ps[:, 0:256], lhsT=iw_t, rhs=E16,
                                     start=False, stop=False,
                                     skip_group_check=True)
                    nc.tensor.matmul(o_ps[:, 256:512], lhsT=iw_t, rhs=E16,
                                     start=False, stop=True,
                                     skip_group_check=True)

        def finish(nrm, chunks=CHUNKS):
            """exp + j-sum + drain + out-DMA for one body's norms."""
            E = Ep.tile([128, 15 * 512 + 256], f16, tag="E")
            o_ps = psO.tile([128, 512], f32, tag="oPs")
            for d0, d1 in chunks:
                exp_chunk(nrm, E, d0, d1)
                jsum_chunk(E, o_ps, d0, d1)
            o_sb = osb.tile([128, 512], f32, tag="osb")
            nc.scalar.activation(
                out=o_sb, in_=o_ps[:, :],
                func=mybir.ActivationFunctionType.Identity,
                bias=1.0, scale=1.0,
            )
            nc.sync.dma_start(out=o_d, in_=o_sb)

        # -- software-pipelined unrolled block --
        m2 = stage_b(load_x(0))
        nrm_prev = None
        for k in range(unroll):
            if k + 1 < unroll:
                nrm = dve_chunks(m2)
                m2 = stage_b(load_x(k + 1))
                if nrm_prev is not None:
                    finish(nrm_prev)
                nrm_prev = nrm
            else:
                # last body: fine DVE chunks with exp/j-sum trailing one
                # chunk so the post-barrier tail is only the final chunk.
                FINE = ((1, 5), (6, 10), (11, 15), (16, 16))
                nrm = nrmp.tile([128, 15 * 512 + 256], f16, tag="nrm")
                E = Ep.tile([128, 15 * 512 + 256], f16, tag="E")
                o_ps = psO.tile([128, 512], f32, tag="oPs")
                for ci, (d0, d1) in enumerate(FINE):
                    dve_chunks(m2, chunks=((d0, d1),), nrm=nrm)
                    if ci == 0 and nrm_prev is not None:
                        finish(nrm_prev)
                    if ci > 0:
                        pd0, pd1 = FINE[ci - 1]
                        exp_chunk(nrm, E, pd0, pd1)
                        jsum_chunk(E, o_ps, pd0, pd1)
                exp_chunk(nrm, E, *FINE[-1])
                jsum_chunk(E, o_ps, *FINE[-1])
                o_sb = osb.tile([128, 512], f32, tag="osb")
                nc.scalar.activation(
                    out=o_sb, in_=o_ps[:, :],
                    func=mybir.ActivationFunctionType.Identity,
                    bias=1.0, scale=1.0,
                )
                nc.sync.dma_start(out=o_d, in_=o_sb)


# --------------------------------------------------------------------------
# host side
# --------------------------------------------------------------------------

def prep_inputs(x, T):
    """Shared (core-independent) device inputs, packed partition-first."""
    xf = np.ascontiguousarray(x.reshape(N, HW, F))
    # tw[f, (fh, c, b)] = T[fh*128+f, b, c]
    tw = T.reshape(FH, 128, B, C).transpose(1, 0, 3, 2)     # f, fh, c, b
    tw_in = np.ascontiguousarray(tw.reshape(128, FH * C * B)).astype(np.float16)
    iw_in = np.eye(128, dtype=np.float16)
    return xf, tw_in, iw_in


def core_in_map(xf, tw_in, iw_in, k):
    xs = xf[:, k * HWL:(k + 1) * HWL, :]          # [i, hwl, f]
    # xT[f, (fh, hh, i, hl)] = x[i, hh*16+hl, fh*128+f]
    xT = xs.reshape(N, HH, HL, FH, 128).transpose(4, 3, 1, 0, 2)
    xT = np.ascontiguousarray(xT.reshape(128, FH * HH * N * HL))
    return {"xT": xT.astype(np.float16), "tw": tw_in, "iw": iw_in}


def gather_ob(core_outs):
    """core_outs: list of 8 arrays [128,512] f32 -> o_b [N,16,16,B]."""
    obs = []
    for res in core_outs:
        v = res.astype(np.float32).reshape(HH, B, N, HL)   # hh, b, i, hl
        obs.append(v.transpose(2, 0, 3, 1).reshape(N, HWL, B))  # i, hwl, b
    return np.concatenate(obs, axis=1).reshape(N, 16, 16, B)


def _get_program(reps=1, loop=None, unroll=UNROLL):
    key = ("nc", reps, loop, unroll)
    if key in _CACHED:
        return _CACHED[key]
    from contextlib import ExitStack
    import concourse.bacc as bacc
    import concourse.mybir as mybir
    import concourse.tile as tile

    _get_ps8_op()
    nc = bacc.Bacc("TRN2", target_bir_lowering=False, debug=False,
                   num_devices=CORES)
    f16, f32 = mybir.dt.float16, mybir.dt.float32
    ins = {
        "xT": nc.dram_tensor("xT", [128, FH * HH * N * HL], f16,
                             kind="ExternalInput").ap(),
        "tw": nc.dram_tensor("tw", [128, FH * C * B], f16,
                             kind="ExternalInput").ap(),
        "iw": nc.dram_tensor("iw", [128, 128], f16,
                             kind="ExternalInput").ap(),
    }
    outs = {
        "o": nc.dram_tensor("o", [128, 512], f32, kind="ExternalOutput").ap(),
    }
    with tile.TileContext(nc) as tc:
        if loop:
            with ExitStack() as ctx:
                pools = make_pools(tc, ctx)
                with tc.For_i(0, loop, 1,
                              hint_engines=(mybir.EngineType.PE,
                                            mybir.EngineType.DVE)):
                    build_block(tc, outs, ins, pools=pools, unroll=unroll)
        else:
            for r in range(reps):
                build_block(tc, outs, ins, rep=r, unroll=1)
    nc.compile()
    _CACHED[key] = nc
    return nc


def kernel(x, T):
    x = np.asarray(x, dtype=np.float32)
    T = np.asarray(T, dtype=np.float32)
    from concourse.bass_utils import run_bass_kernel_spmd

    nc = _get_program()
    xf, tw_in, iw_in = prep_inputs(x, T)
    in_maps = [core_in_map(xf, tw_in, iw_in, k) for k in range(CORES)]
    res = run_bass_kernel_spmd(nc, in_maps, core_ids=list(range(CORES)))
    ob = gather_ob([r["o"] for r in res.results])
    return np.concatenate([x, ob], axis=3)
